# revision 19
# baseline (speedup 1.0000x reference)
"""Trainium2 Bass kernel for nn_AnalysisTransform (3D sparse conv encoder).

Pipeline (per batch): conv5^3/s2 -> GDN -> conv/s2 -> GDN -> conv/s2 -> GDN
-> conv/s1, with voxel masks and a per-batch conditional scale.

Distribution over 8 NeuronCores: core c handles batch b=c//4, z-slab s=c%4.
Stage A (conv1 + GDN1 at 32^3) is z-sharded 4-way per batch; the GDN1 output
is AllGathered within each 4-core group; the remaining layers (16^3 and 8^3)
run replicated per group (cores 0 and 4 provide the outputs).

All matmuls run in bf16 with f32 PSUM accumulation. Convs are shift-matmuls:
contraction packs Cin x 4 z-offsets (K<=128) via z-shifted SBUF replicas;
conv1 uses host-side im2col over (dy,dx) giving K=100. The per-batch GDN
scale q[b,0] is folded into gamma/beta on the host (y = x*rsqrt((beta+den)/s^2)).

q (mean of Q_F) and k (mask voxel counts) are tiny reductions done on host.
"""
import os
import numpy as np
import ml_dtypes

import concourse.bass as bass
import concourse.bacc as bacc
import concourse.tile as tile
import concourse.mybir as mybir
from concourse.bass_utils import run_bass_kernel_spmd

BF16 = mybir.dt.bfloat16
F32 = mybir.dt.float32
BFNP = ml_dtypes.bfloat16

B = 2

_CACHED_NC = [None]


def _mmgroup(nc, acc, pairs, tile_position=None):
    """Issue an accumulation group of matmuls: pairs = [(lhsT, rhs), ...]."""
    n = len(pairs)
    for i, (lhsT, rhs) in enumerate(pairs):
        nc.tensor.matmul(acc, lhsT, rhs, start=(i == 0), stop=(i == n - 1),
                         tile_position=tile_position)


def _build_nc():
    if _CACHED_NC[0] is not None:
        return _CACHED_NC[0]
    nc = bacc.Bacc("TRN2", target_bir_lowering=False, debug=False, num_devices=8)
    AFT = mybir.ActivationFunctionType
    ALU = mybir.AluOpType

    dp = nc.declare_dram_parameter
    # stage A inputs
    Rd = dp("R", [100, 28, 32, 32], BF16, isOutput=False)
    m1d = dp("m1", [32, 12, 32, 32], BF16, isOutput=False)
    W1d = dp("W1T", [100, 5, 32], BF16, isOutput=False)
    # combined weights: [:, 0:25] (or 0:50 for paired) = packed-K part on 128
    # partitions; trailing 25 cols = dz=4 leftover on the first Cin partitions.
    G1Ad = dp("G1A", [128, 5, 160], BF16, isOutput=False)
    G1Bd = dp("G1B", [32, 5, 160], BF16, isOutput=False)
    W2d = dp("W2", [128, 50, 64], BF16, isOutput=False)
    G2PAd = dp("G2PA", [128, 2, 5, 128], BF16, isOutput=False)
    G2PBd = dp("G2PB", [128, 2, 5, 128], BF16, isOutput=False)
    G2E4d = dp("G2E4", [64, 5, 256], BF16, isOutput=False)
    G2Xd = dp("G2X", [128, 2, 5, 64], BF16, isOutput=False)
    G2X4d = dp("G2X4", [64, 5, 64], BF16, isOutput=False)
    W3d = dp("W3", [128, 75, 96], BF16, isOutput=False)
    G3d = dp("G3", [96, 125, 96], BF16, isOutput=False)
    W4d = dp("W4", [96, 125, 128], BF16, isOutput=False)
    # biases / betas (beta already divided by s^2)
    b1d = dp("b1", [128, 1], F32, isOutput=False)
    b2d = dp("b2", [128, 1], F32, isOutput=False)
    b3d = dp("b3", [96, 1], F32, isOutput=False)
    b4d = dp("b4", [128, 1], F32, isOutput=False)
    t1d = dp("bt1", [128, 1], F32, isOutput=False)
    t2d = dp("bt2", [128, 1], F32, isOutput=False)
    t3d = dp("bt3", [96, 1], F32, isOutput=False)
    # masks
    m2d = dp("m2", [64, 16 * 16 * 16], BF16, isOutput=False)
    m3ad = dp("m3a", [96, 512], BF16, isOutput=False)
    m3bd = dp("m3b", [128, 512], BF16, isOutput=False)
    zd = dp("zeros", [128, 12960], BF16, isOutput=False)

    outd = dp("out", [128, 512], F32, isOutput=True)

    P1 = 1296  # 36*36 plane
    P2 = 400   # 20*20 plane
    P3 = 144   # 12*12 plane
    cc1_ins = [nc.dram_tensor(f"cc1_in{i}", [32, 2 * P1], BF16) for i in range(4)]
    cc1_outs = [nc.dram_tensor(f"cc1_out{i}", [4, 32, 2 * P1], BF16) for i in range(4)]

    with tile.TileContext(nc) as tc:
        with (
            tc.tile_pool(name="persist", bufs=1) as pp,
            tc.tile_pool(name="stream", bufs=2) as st,
            tc.tile_pool(name="win", bufs=2) as wpool,
            tc.tile_pool(name="ps", bufs=8, space="PSUM") as ps,
        ):
            def load(pool, dram, shape, dtype=BF16, tag=None):
                t = pool.tile(shape, dtype, name=dram.name + "_t", tag=tag or "")
                nc.sync.dma_start(t[:], dram[:])
                return t

            W1t = load(pp, W1d, [100, 5, 32])
            G1At = load(pp, G1Ad, [128, 5, 160])
            G1Bt = load(pp, G1Bd, [32, 5, 160])
            # R first chunks next (feeds the first matmuls)
            R0t = pp.tile([100, 14, 32, 32], BF16, name="R0t", tag="rA")
            for i in range(4):
                a, b = (0, 4, 8, 11)[i], (4, 8, 11, 14)[i]
                nc.sync.dma_start(R0t[:, a:b], Rd[:, a:b])
            R1t = pp.tile([100, 14, 32, 32], BF16, name="R1t", tag="rB")
            for i in range(4):
                a, b = (0, 4, 8, 11)[i], (4, 8, 11, 14)[i]
                nc.sync.dma_start(R1t[:, a:b], Rd[:, 14 + a : 14 + b])
            b1t = load(pp, b1d, [128, 1], F32)
            b2t = load(pp, b2d, [128, 1], F32)
            b3t = load(pp, b3d, [96, 1], F32)
            b4t = load(pp, b4d, [128, 1], F32)
            t1t = load(pp, t1d, [128, 1], F32)
            t2t = load(pp, t2d, [128, 1], F32)
            t3t = load(pp, t3d, [96, 1], F32)
            m3at = load(pp, m3ad, [96, 512])
            m3bt = load(pp, m3bd, [128, 512])

            def Rplane(zi):  # [100, 32, 32] view of input plane zi
                return (R0t if zi < 14 else R1t)[:, zi % 14, :, :]

            # XM1 packed: partition group g=zi//3 holds planes 3g..3g+3
            XM1p = pp.tile([128, 3 * P1], BF16, name="XM1p")
            nc.sync.dma_start(XM1p[:], zd[:, 0 : 3 * P1])

            def xm1_plane(zi):  # [32, 36, 36] view of slab plane zi in [0,12)
                g, r = divmod(zi, 3)
                v = XM1p[32 * g : 32 * g + 32, r * P1 : (r + 1) * P1]
                return v.rearrange("p (y x) -> p y x", y=36)

            for i in range(4):
                nc.sync.dma_start(cc1_ins[i][:], zd[0:32, 0 : 2 * P1])

            # ---- L1: conv1 s2, out slab zeta in [0,12) -> XM1 interior ----
            for zi in range(12):
                pb = 32 * (zi // 3)
                for h in range(2):
                    acc = ps.tile([128, 512], F32, name="acc", tag="acc")
                    _mmgroup(nc, acc[pb : pb + 32, :], [
                        (W1t[:, dz, :], Rplane(2 * zi + dz)[:, 16 * h : 16 * h + 16, :])
                        for dz in range(5)
                    ], tile_position=(0, pb))
                    m1c = st.tile([128, 512], BF16, name="m1c", tag="m1c")
                    nc.sync.dma_start(m1c[pb : pb + 32, :], m1d[:, zi, 16 * h : 16 * h + 16, :])
                    av = acc[pb : pb + 32, :].rearrange("p (y x) -> p y x", y=16)
                    nc.vector.scalar_tensor_tensor(
                        xm1_plane(zi)[:, 2 + 16 * h : 18 + 16 * h, 2:34],
                        av, b1t[pb : pb + 32, :],
                        m1c[pb : pb + 32, :].rearrange("p (y x) -> p y x", y=16),
                        ALU.add, ALU.mult,
                    )

            # ---- SQ1 (packed, one full-width mul) + z-shift replicas ----
            SQ1p = pp.tile([128, 3 * P1], BF16, name="SQ1p")
            def sq1_plane(zi):
                g, r = divmod(zi, 3)
                return SQ1p[32 * g : 32 * g + 32, r * P1 : (r + 1) * P1]
            for u in range(12):
                nc.vector.tensor_mul(sq1_plane(u), xm1_plane(u).rearrange("p y x -> p (y x)"), xm1_plane(u).rearrange("p y x -> p (y x)"))
            REP1 = pp.tile([128, 12 * P1], BF16, name="REP1", tag="rep")
            # REP1[32g+c, t] = sq[t+g-2] = SQ1p plane u=t+g
            for g in range(4):
                u = g
                while u < 12:
                    ug, ur = divmod(u, 3)
                    run = min(3 - ur, 12 - u)
                    nc.sync.dma_start(
                        REP1[32 * g : 32 * g + 32, (u - g) * P1 : (u - g + run) * P1],
                        SQ1p[32 * ug : 32 * ug + 32, ur * P1 : (ur + run) * P1],
                    )
                    u += run
            R1view = REP1[:].rearrange("p (z y x) -> p z y x", z=12, y=36)

            # ---- GDN1: den conv (M-packed over dx) + y1 -> cc1_in ----
            R1flat = REP1[:].rearrange("p (z c) -> p z c", z=12)
            for zi in range(8):
                pb = 32 * ((zi + 2) // 3)
                for (y0, cy) in ((0, 14), (14, 14), (28, 4)):
                    NN = cy * 36
                    pmm = ps.tile([128, 504], F32, name="pmm", tag="acc")
                    pv = pmm[:, 0:NN]
                    # matmuls sorted by (K, M) geometry: mixing weight
                    # geometries within a stream costs ~35% PE throughput
                    mms = []
                    NE = NN - 4
                    for dy in range(5):
                        row = (y0 + dy) * 36
                        mms.append((pv, G1At[:, dy, 0:128], R1flat[:, zi, row : row + NN]))
                    for dy in range(5):
                        row = (y0 + dy) * 36
                        mms.append((pv, G1Bt[:, dy, 0:128], R1flat[0:32, zi + 4, row : row + NN]))
                    for dy in range(5):
                        row = (y0 + dy) * 36
                        # edge (dx=4): contiguous flat read shifted +4; trailing
                        # cols of each psum row take garbage the combine skips
                        mms.append((pmm[0:32, 0:NE], G1At[:, dy, 128:160],
                                    R1flat[:, zi, row + 4 : row + 4 + NE]))
                    for dy in range(5):
                        row = (y0 + dy) * 36
                        mms.append((pmm[0:32, 0:NE], G1Bt[:, dy, 128:160],
                                    R1flat[0:32, zi + 4, row + 4 : row + 4 + NE]))
                    for i, (out_ap, lhsT, rhs) in enumerate(mms):
                        nc.tensor.matmul(out_ap, lhsT, rhs,
                                         start=(i == 0), stop=(i == len(mms) - 1),
                                         skip_group_check=True)
                    pvv = pv.rearrange("p (y x) -> p y x", y=cy)
                    comb = st.tile([128, 896], F32, name="comb", tag="comb")
                    s1 = comb[pb : pb + 32, 0 : cy * 32]
                    s1v = s1.rearrange("p (y x) -> p y x", y=cy)
                    nc.scalar.activation(s1v, pvv[0:32, :, 0:32], AFT.Copy)
                    nc.vector.scalar_tensor_tensor(
                        s1v, pvv[32:64, :, 1:33], 0.0, s1v, ALU.add, ALU.add)
                    nc.vector.scalar_tensor_tensor(
                        s1v, pvv[64:96, :, 2:34], 0.0, s1v, ALU.add, ALU.add)
                    nc.vector.scalar_tensor_tensor(
                        s1v, pvv[96:128, :, 3:35], t1t[pb : pb + 32, :], s1v,
                        ALU.add, ALU.add)
                    tt = st.tile([128, 512], F32, name="tt", tag="tden")
                    nc.scalar.activation(tt[pb : pb + 32, 0 : cy * 32], s1, AFT.Sqrt)
                    rr = st.tile([128, 512], F32, name="rr", tag="rden")
                    nc.vector.reciprocal(rr[pb : pb + 32, 0 : cy * 32],
                                         tt[pb : pb + 32, 0 : cy * 32])
                    y1c = st.tile([128, 512], BF16, name="y1c", tag="y1c")
                    nc.vector.tensor_mul(
                        y1c[pb : pb + 32, 0 : cy * 32].rearrange("p (y x) -> p y x", y=cy),
                        rr[pb : pb + 32, 0 : cy * 32].rearrange("p (y x) -> p y x", y=cy),
                        xm1_plane(zi + 2)[:, 2 + y0 : 2 + y0 + cy, 2:34],
                    )
                    cct = cc1_ins[zi // 2]
                    cv = cct[:].rearrange("p (z y x) -> p z y x", z=2, y=36)
                    nc.sync.dma_start(
                        cv[:, zi % 2, 2 + y0 : 2 + y0 + cy, 2:34],
                        y1c[pb : pb + 32, 0 : cy * 32].rearrange("p (y x) -> p y x", y=cy),
                    )

            W2t = load(pp, W2d, [128, 50, 64])
            G2PAt = load(pp, G2PAd, [128, 2, 5, 128])
            G2PBt = load(pp, G2PBd, [128, 2, 5, 128])
            G2E4t = load(pp, G2E4d, [64, 5, 256])
            G2Xt = load(pp, G2Xd, [128, 2, 5, 64])
            G2X4t = load(pp, G2X4d, [64, 5, 64])
            W3t = load(pp, W3d, [128, 75, 96])
            G3t = load(pp, G3d, [96, 125, 96], tag="rA")
            W4t = load(pp, W4d, [96, 125, 128], tag="rB")

            # ---- AG1 (split in two so the first half overlaps GDN1) ----
            for i in range(4):
                nc.gpsimd.collective_compute(
                    "AllGather", mybir.AluOpType.bypass,
                    replica_groups=[[0, 1, 2, 3], [4, 5, 6, 7]],
                    ins=[cc1_ins[i][:]], outs=[cc1_outs[i][:]],
                )

            # ---- Stage B (full batch per core) ----
            # XM2/Y2/SQ2 packed: partition group g=zp//10 holds planes 10g..10g+10
            XM2p = pp.tile([128, 10 * P2], BF16, name="XM2p")
            nc.sync.dma_start(XM2p[:], zd[:, 0 : 10 * P2])
            Y2p = pp.tile([128, 10 * P2], BF16, name="Y2p")
            nc.sync.dma_start(Y2p[:], zd[:, 0 : 10 * P2])

            def pk2(tilep, zp, n):  # [64, n, 20, 20] view, planes zp..zp+n (same decade)
                g, r = divmod(zp, 10)
                assert r + n <= 10
                v = tilep[64 * g : 64 * g + 64, r * P2 : (r + n) * P2]
                return v.rearrange("p (z y x) -> p z y x", z=n, y=20)

            # L2: conv2 s2 from gathered y1 (windowed z planes)
            # wt[g][col d] = y1pad[2*(z2+d)+g]; wt4[0:32][col d] = y1pad[2*(z2+d)+4]
            def fill_plane(dst, pcol, zp):
                """DMA y1pad plane zp (global padded z) into dst plane column."""
                if zp < 2 or zp >= 34:
                    nc.sync.dma_start(dst[:, pcol * P1 : (pcol + 1) * P1], zd[0:32, 0:P1])
                else:
                    rank, zz = (zp - 2) // 8, (zp - 2) % 8
                    cct = cc1_outs[zz // 2]
                    nc.sync.dma_start(
                        dst[:, pcol * P1 : (pcol + 1) * P1],
                        cct[rank, :, (zz % 2) * P1 : (zz % 2 + 1) * P1],
                    )

            for z2 in range(0, 16, 2):
                wt = wpool.tile([128, 2 * P1], BF16, name="wt", tag="l2win")
                wt4 = wpool.tile([32, 2 * P1], BF16, name="wt4", tag="l2win4")
                for g in range(4):
                    for d in range(2):
                        fill_plane(wt[32 * g : 32 * g + 32], d, 2 * (z2 + d) + g)
                for d in range(2):
                    fill_plane(wt4[0:32], d, 2 * (z2 + d) + 4)
                wv = wt[:].rearrange("p (z y x) -> p z y x", z=2, y=36)
                w4v = wt4[:].rearrange("p (z y x) -> p z y x", z=2, y=36)
                pb = 64 * ((2 + z2) // 10)
                acc = ps.tile([128, 512], F32, name="acc", tag="acc")
                mms = []
                for dy in range(5):
                    for dx in range(5):
                        mms.append((
                            W2t[:, dy * 5 + dx, :],
                            wv[:, :, dy : dy + 31 : 2, dx : dx + 31 : 2],
                        ))
                for dy in range(5):
                    for dx in range(5):
                        mms.append((
                            W2t[0:32, 25 + dy * 5 + dx, :],
                            w4v[:, :, dy : dy + 31 : 2, dx : dx + 31 : 2],
                        ))
                _mmgroup(nc, acc[pb : pb + 64, :], mms, tile_position=(0, pb))
                m2c = st.tile([128, 512], BF16, name="m2c", tag="m2c")
                nc.sync.dma_start(m2c[pb : pb + 64, :], m2d[:, z2 * 256 : (z2 + 2) * 256])
                for d in range(2):
                    av = acc[pb : pb + 64, 256 * d : 256 * d + 256].rearrange(
                        "p (y x) -> p y x", y=16)
                    nc.vector.scalar_tensor_tensor(
                        pk2(XM2p, 2 + z2 + d, 1)[:, 0, 2:18, 2:18],
                        av, b2t[pb : pb + 64, :],
                        m2c[pb : pb + 64, 256 * d : 256 * d + 256].rearrange(
                            "p (y x) -> p y x", y=16),
                        ALU.add, ALU.mult,
                    )

            # SQ2 packed + replicas (pz=2)
            SQ2p = pp.tile([128, 10 * P2], BF16, name="SQ2p")
            for zp in range(20):
                g, r = divmod(zp, 10)
                nc.vector.tensor_mul(
                    SQ2p[64 * g : 64 * g + 64, r * P2 : (r + 1) * P2],
                    XM2p[64 * g : 64 * g + 64, r * P2 : (r + 1) * P2],
                    XM2p[64 * g : 64 * g + 64, r * P2 : (r + 1) * P2])

            def build_rep2(dst, srcp):
                # dst[0:64][tz] = src[tz]; dst[64:128][tz] = src[tz+1]
                for a, b in ((0, 5), (5, 10)):
                    nc.sync.dma_start(dst[0:64, a * P2 : b * P2],
                                      srcp[0:64, a * P2 : b * P2])
                    nc.sync.dma_start(dst[0:64, (10 + a) * P2 : (10 + b) * P2],
                                      srcp[64:128, a * P2 : b * P2])
                for a, b in ((0, 5), (5, 9)):
                    nc.sync.dma_start(dst[64:128, a * P2 : b * P2],
                                      srcp[0:64, (a + 1) * P2 : (b + 1) * P2])
                nc.sync.dma_start(dst[64:128, 9 * P2 : 10 * P2], srcp[64:128, 0:P2])
                for a, b in ((0, 5), (5, 9)):
                    nc.sync.dma_start(dst[64:128, (10 + a) * P2 : (10 + b) * P2],
                                      srcp[64:128, (a + 1) * P2 : (b + 1) * P2])

            REP2 = pp.tile([128, 20 * P2], BF16, name="REP2", tag="rep")
            build_rep2(REP2, SQ2p)
            R2v = REP2[:].rearrange("p (z y x) -> p z y x", z=20, y=20)

            # GDN2 (M-packed over dx pairs, full-row psum, contiguous rhs)
            R2flat = REP2[:].rearrange("p (z c) -> p z c", z=20)
            for z2 in range(16):
                pb = 64 * ((2 + z2) // 10)
                psA = ps.tile([128, 320], F32, name="psA", tag="acc")
                psB = ps.tile([128, 320], F32, name="psB", tag="acc")
                mmsA, mmsB = [], []
                for Kp in range(2):
                    for dy in range(5):
                        rhs = R2flat[:, z2 + 2 * Kp, dy * 20 : dy * 20 + 320]
                        mmsA.append((psA[:], G2PAt[:, Kp, dy, :], rhs))
                        mmsB.append((psB[:], G2PBt[:, Kp, dy, :], rhs))
                for dy in range(5):
                    rhs = R2flat[0:64, z2 + 4, dy * 20 : dy * 20 + 320]
                    mmsA.append((psA[:], G2E4t[:, dy, 0:128], rhs))
                    mmsB.append((psB[:], G2E4t[:, dy, 128:256], rhs))
                for Kp in range(2):
                    for dy in range(5):
                        rhs_e = R2flat[:, z2 + 2 * Kp, dy * 20 + 4 : dy * 20 + 320]
                        mmsA.append((psA[0:64, 0:316], G2Xt[:, Kp, dy, :], rhs_e))
                for dy in range(5):
                    rhs_e = R2flat[0:64, z2 + 4, dy * 20 + 4 : dy * 20 + 320]
                    mmsA.append((psA[0:64, 0:316], G2X4t[:, dy, :], rhs_e))
                for grp in (mmsA, mmsB):
                    for i, (out_ap, lhsT, rhs) in enumerate(grp):
                        nc.tensor.matmul(out_ap, lhsT, rhs,
                                         start=(i == 0), stop=(i == len(grp) - 1),
                                         skip_group_check=True)
                pAv = psA[:].rearrange("p (y x) -> p y x", y=16)
                pBv = psB[:].rearrange("p (y x) -> p y x", y=16)
                comb = st.tile([128, 896], F32, name="comb", tag="comb")
                s1 = comb[pb : pb + 64, 0:256]
                s1v = s1.rearrange("p (y x) -> p y x", y=16)
                nc.scalar.activation(s1v, pAv[0:64, :, 0:16], AFT.Copy)
                nc.vector.scalar_tensor_tensor(
                    s1v, pAv[64:128, :, 1:17], 0.0, s1v, ALU.add, ALU.add)
                nc.vector.scalar_tensor_tensor(
                    s1v, pBv[0:64, :, 2:18], 0.0, s1v, ALU.add, ALU.add)
                nc.vector.scalar_tensor_tensor(
                    s1v, pBv[64:128, :, 3:19], t2t[pb : pb + 64, :], s1v,
                    ALU.add, ALU.add)
                tt = st.tile([128, 512], F32, name="tt", tag="tden")
                nc.scalar.activation(tt[pb : pb + 64, 0:256], s1, AFT.Sqrt)
                rr = st.tile([128, 512], F32, name="rr", tag="rden")
                nc.vector.reciprocal(rr[pb : pb + 64, 0:256], tt[pb : pb + 64, 0:256])
                rv = rr[pb : pb + 64, 0:256].rearrange("p (y x) -> p y x", y=16)
                nc.vector.tensor_mul(
                    pk2(Y2p, 2 + z2, 1)[:, 0, 2:18, 2:18],
                    rv, pk2(XM2p, 2 + z2, 1)[:, 0, 2:18, 2:18],
                )

            # L3 replicas from Y2 (pz=2)
            REP3 = pp.tile([128, 20 * P2], BF16, name="REP3", tag="rep")
            build_rep2(REP3, Y2p)
            R3v = REP3[:].rearrange("p (z y x) -> p z y x", z=20, y=20)

            # L3: conv3 s2, full 8^3 volume in one psum tile
            acc3 = ps.tile([96, 512], F32, name="acc3", tag="acc")
            mms = []
            for p in range(2):
                for dy in range(5):
                    for dx in range(5):
                        mms.append((
                            W3t[:, p * 25 + dy * 5 + dx, :],
                            R3v[:, 2 * p : 2 * p + 15 : 2, dy : dy + 15 : 2, dx : dx + 15 : 2],
                        ))
            for dy in range(5):
                for dx in range(5):
                    mms.append((
                        W3t[0:64, 50 + dy * 5 + dx, :],
                        R3v[0:64, 4 : 4 + 15 : 2, dy : dy + 15 : 2, dx : dx + 15 : 2],
                    ))
            _mmgroup(nc, acc3[:], mms)
            XM3 = pp.tile([96, 12 * P3], BF16, name="XM3", tag="XM2p")
            nc.sync.dma_start(XM3[:], zd[0:96, 0 : 12 * P3])
            XM3v = XM3[:].rearrange("p (z y x) -> p z y x", z=12, y=12)
            for z in range(8):
                a3v = acc3[:, 64 * z : 64 * z + 64].rearrange("p (y x) -> p y x", y=8)
                m3av = m3at[:, 64 * z : 64 * z + 64].rearrange("p (y x) -> p y x", y=8)
                nc.vector.scalar_tensor_tensor(
                    XM3v[:, 2 + z, 2:10, 2:10], a3v, b3t[:], m3av, ALU.add, ALU.mult
                )

            SQ3 = pp.tile([96, 12 * P3], BF16, name="SQ3", tag="SQ2p")
            nc.vector.tensor_mul(SQ3[:], XM3[:], XM3[:])
            S3v = SQ3[:].rearrange("p (z y x) -> p z y x", z=12, y=12)

            # GDN3 (K=96, no packing)
            acc4 = ps.tile([96, 512], F32, name="acc4", tag="acc")
            mms = []
            for o in range(125):
                dz, r = divmod(o, 25)
                dy, dx = divmod(r, 5)
                mms.append((G3t[:, o, :], S3v[:, dz : dz + 8, dy : dy + 8, dx : dx + 8]))
            _mmgroup(nc, acc4[:], mms)
            tt3 = st.tile([96, 512], F32, name="tt3", tag="tden")
            nc.scalar.activation(tt3[:], acc4[:], AFT.Sqrt, bias=t3t[:])
            rr3 = st.tile([96, 512], F32, name="rr3", tag="rden")
            nc.vector.reciprocal(rr3[:], tt3[:])
            Y3 = pp.tile([96, 12 * P3], BF16, name="Y3", tag="Y2p")
            nc.sync.dma_start(Y3[:], zd[0:96, 0 : 12 * P3])
            Y3v = Y3[:].rearrange("p (z y x) -> p z y x", z=12, y=12)
            for z in range(8):
                r3v = rr3[:, 64 * z : 64 * z + 64].rearrange("p (y x) -> p y x", y=8)
                nc.vector.tensor_mul(
                    Y3v[:, 2 + z, 2:10, 2:10], r3v, XM3v[:, 2 + z, 2:10, 2:10]
                )

            # L4: conv4 s1
            acc5 = ps.tile([128, 512], F32, name="acc5", tag="acc")
            mms = []
            for o in range(125):
                dz, r = divmod(o, 25)
                dy, dx = divmod(r, 5)
                mms.append((W4t[:, o, :], Y3v[:, dz : dz + 8, dy : dy + 8, dx : dx + 8]))
            _mmgroup(nc, acc5[:], mms)
            outt = st.tile([128, 512], F32, name="outt", tag="outt", bufs=1)
            nc.vector.scalar_tensor_tensor(
                outt[:], acc5[:], b4t[:], m3bt[:], ALU.add, ALU.mult
            )
            nc.sync.dma_start(outd[:], outt[:])

    nc.compile()
    _CACHED_NC[0] = nc
    return nc


def _prep_core_inputs(c, x_feat, m1f, m2f, m3f, scale, weights):
    """Build the per-core input map. c in [0,8): batch c//4, slab c%4."""
    b, s = divmod(c, 4)
    (w1, b1, w2, b2, w3, b3, w4, b4, be1, ga1, be2, ga2, be3, ga3) = weights
    s2 = float(scale[b]) ** 2

    # R: im2col of padded input for out-z zeta in [0,12) (z1 = 8s-2+zeta)
    zp6 = np.pad(x_feat[b], ((0, 0), (6, 6), (2, 2), (2, 2)))  # [4, 76, 68, 68]
    zs = 16 * s
    sl = zp6[:, zs : zs + 28]  # [4, 28, 68, 68]
    Rarr = np.empty((4, 5, 5, 28, 32, 32), np.float32)
    for dy in range(5):
        for dx in range(5):
            Rarr[:, dy, dx] = sl[:, :, dy : dy + 63 : 2, dx : dx + 63 : 2]
    R = Rarr.reshape(100, 28, 32, 32).astype(BFNP)

    # m1 slab: z1 in [8s-2, 8s+10), replicated to 32 channels
    m1s = np.zeros((12, 32, 32), np.float32)
    lo, hi = 8 * s - 2, 8 * s + 10
    clo, chi = max(lo, 0), min(hi, 32)
    m1s[clo - lo : chi - lo] = m1f[b, clo:chi]
    m1 = np.broadcast_to(m1s, (32, 12, 32, 32)).astype(BFNP)

    def cw(w):  # [co,ci,dz,dy,dx] -> [dz, ci, dy, dx, co]
        return np.transpose(w, (2, 1, 3, 4, 0))

    W1T = np.transpose(w1, (1, 3, 4, 2, 0)).reshape(100, 5, 32).astype(BFNP)

    def packed_pairs(t, ci, co):
        # t [5, ci, 5, 5, co] -> [2*ci, 2*25, co] for dz pairs (2p, 2p+1)
        out = np.zeros((2, ci, 2, 25, co), np.float32)
        for j in range(2):
            for p in range(2):
                out[j, :, p] = t[2 * p + j].reshape(ci, 25, co)
        return out.reshape(2 * ci, 50, co)

    def combine(main, edge, ci_edge, co):
        # main [<=128, ncols, co]; edge [ci_edge, 25, co]
        ncols = main.shape[1]
        full = np.zeros((128, ncols + 25, co), np.float32)
        full[: main.shape[0], :ncols] = main
        full[:ci_edge, ncols:] = edge
        return full.astype(BFNP)

    g1 = cw(ga1 / s2)  # [5, 32, 5, 5, 32]
    G1A = np.zeros((128, 5, 160), np.float32)
    for dz in range(4):
        for dx in range(4):
            G1A[32 * dz : 32 * dz + 32, :, 32 * dx : 32 * dx + 32] = g1[dz][:, :, dx, :]
        G1A[32 * dz : 32 * dz + 32, :, 128:160] = g1[dz][:, :, 4, :]
    G1A = G1A.astype(BFNP)
    G1B = np.zeros((32, 5, 160), np.float32)
    for dx in range(4):
        G1B[:, :, 32 * dx : 32 * dx + 32] = g1[4][:, :, dx, :]
    G1B[:, :, 128:160] = g1[4][:, :, 4, :]
    G1B = G1B.astype(BFNP)

    t2 = cw(w2)
    W2main = np.concatenate([t2[dz].reshape(32, 25, 64) for dz in range(4)], axis=0)
    W2 = combine(W2main, t2[4].reshape(32, 25, 64), 32, 64)

    g2 = cw(ga2 / s2)  # [5, 64, 5, 5, 64]
    G2PA = np.zeros((128, 2, 5, 128), np.float32)
    G2PB = np.zeros((128, 2, 5, 128), np.float32)
    G2X = np.zeros((128, 2, 5, 64), np.float32)
    for jz in range(2):
        for Kp in range(2):
            dz = 2 * Kp + jz
            for jx in range(2):
                G2PA[64 * jz : 64 * jz + 64, Kp, :, 64 * jx : 64 * jx + 64] = g2[dz][:, :, jx, :]
                G2PB[64 * jz : 64 * jz + 64, Kp, :, 64 * jx : 64 * jx + 64] = g2[dz][:, :, 2 + jx, :]
            G2X[64 * jz : 64 * jz + 64, Kp, :, :] = g2[dz][:, :, 4, :]
    G2E4 = np.zeros((64, 5, 256), np.float32)
    for jx in range(2):
        G2E4[:, :, 64 * jx : 64 * jx + 64] = g2[4][:, :, jx, :]
        G2E4[:, :, 128 + 64 * jx : 192 + 64 * jx] = g2[4][:, :, 2 + jx, :]
    G2X4 = g2[4][:, :, 4, :]
    G2PA, G2PB, G2X, G2E4, G2X4 = (a.astype(BFNP) for a in (G2PA, G2PB, G2X, G2E4, G2X4))

    t3 = cw(w3)
    W3 = combine(packed_pairs(t3, 64, 96), t3[4].reshape(64, 25, 96), 64, 96)

    G3 = np.transpose(ga3 / s2, (1, 2, 3, 4, 0)).reshape(96, 125, 96).astype(BFNP)
    W4 = np.transpose(w4, (1, 2, 3, 4, 0)).reshape(96, 125, 128).astype(BFNP)

    m2 = np.broadcast_to(m2f[b].reshape(1, -1), (64, 4096)).astype(BFNP)
    m3a = np.broadcast_to(m3f[b].reshape(1, -1), (96, 512)).astype(BFNP)
    m3b = np.broadcast_to(m3f[b].reshape(1, -1), (128, 512)).astype(BFNP)

    return dict(
        R=np.ascontiguousarray(R), m1=np.ascontiguousarray(m1),
        W1T=W1T, G1A=G1A, G1B=G1B, W2=W2, W3=W3, G3=G3, W4=W4,
        G2PA=np.ascontiguousarray(G2PA), G2PB=np.ascontiguousarray(G2PB),
        G2E4=np.ascontiguousarray(G2E4), G2X=np.ascontiguousarray(G2X),
        G2X4=np.ascontiguousarray(G2X4),
        b1=np.tile(b1.reshape(32, 1), (4, 1)).astype(np.float32),
        b2=np.tile(b2.reshape(64, 1), (2, 1)).astype(np.float32),
        b3=b3.reshape(96, 1).astype(np.float32),
        b4=b4.reshape(128, 1).astype(np.float32),
        bt1=np.tile((be1 / s2).reshape(32, 1), (4, 1)).astype(np.float32),
        bt2=np.tile((be2 / s2).reshape(64, 1), (2, 1)).astype(np.float32),
        bt3=(be3 / s2).reshape(96, 1).astype(np.float32),
        m2=np.ascontiguousarray(m2), m3a=np.ascontiguousarray(m3a),
        m3b=np.ascontiguousarray(m3b),
        zeros=np.zeros((128, 12960), BFNP),
    )


def kernel(x_feat, mask, Q_F, w1, b1, w2, b2, w3, b3, w4, b4,
           beta1, gamma1, beta2, gamma2, beta3, gamma3):
    x_feat = np.asarray(x_feat, np.float32)
    maskf = np.asarray(mask)
    Q_F = np.asarray(Q_F, np.float32)
    args = [np.asarray(a, np.float32) for a in
            (w1, b1, w2, b2, w3, b3, w4, b4, beta1, gamma1, beta2, gamma2, beta3, gamma3)]
    (w1, b1, w2, b2, w3, b3, w4, b4, beta1, gamma1, beta2, gamma2, beta3, gamma3) = args

    # host-side tiny reductions (q, k) and mask pyramid
    m0 = maskf.astype(np.float32)
    q = np.mean(Q_F, axis=1)[None]          # [1, B, 2]
    scale = q[0, :, 0]                      # [B]
    c0 = m0.sum(axis=(1, 2, 3))
    m1f = m0.reshape(B, 32, 2, 32, 2, 32, 2).max(axis=(2, 4, 6))
    c1 = m1f.sum(axis=(1, 2, 3))
    m2f = m1f.reshape(B, 16, 2, 16, 2, 16, 2).max(axis=(2, 4, 6))
    c2 = m2f.sum(axis=(1, 2, 3))
    m3f = m2f.reshape(B, 8, 2, 8, 2, 8, 2).max(axis=(2, 4, 6))
    k = np.stack([c2, c1, c0]).astype(np.int32)

    weights = (w1, b1, w2, b2, w3, b3, w4, b4, beta1, gamma1, beta2, gamma2, beta3, gamma3)
    nc = _build_nc()
    in_maps = [_prep_core_inputs(c, x_feat, m1f, m2f, m3f, scale, weights)
               for c in range(8)]

    trace = os.environ.get("BASS_KERNEL_TRACE", "0") == "1"
    res = run_bass_kernel_spmd(nc, in_maps, core_ids=list(range(8)), trace=trace)
    if trace:
        kernel.last_exec_time_ns = res.exec_time_ns

    x_out = np.stack([
        res.results[0]["out"].reshape(128, 8, 8, 8),
        res.results[4]["out"].reshape(128, 8, 8, 8),
    ]).astype(np.float32)
    return x_out, q.astype(np.float32), k


kernel.last_exec_time_ns = None


# revision 20
# speedup vs baseline: 1.0213x; 1.0213x over previous
"""Trainium2 Bass kernel for nn_AnalysisTransform (3D sparse conv encoder).

Pipeline (per batch): conv5^3/s2 -> GDN -> conv/s2 -> GDN -> conv/s2 -> GDN
-> conv/s1, with voxel masks and a per-batch conditional scale.

Distribution over 8 NeuronCores: core c handles batch b=c//4, z-slab s=c%4.
Stage A (conv1 + GDN1 at 32^3) is z-sharded 4-way per batch; the GDN1 output
is AllGathered within each 4-core group; the remaining layers (16^3 and 8^3)
run replicated per group (cores 0 and 4 provide the outputs).

All matmuls run in bf16 with f32 PSUM accumulation. Convs are shift-matmuls:
contraction packs Cin x 4 z-offsets (K<=128) via z-shifted SBUF replicas;
conv1 uses host-side im2col over (dy,dx) giving K=100. The per-batch GDN
scale q[b,0] is folded into gamma/beta on the host (y = x*rsqrt((beta+den)/s^2)).

q (mean of Q_F) and k (mask voxel counts) are tiny reductions done on host.
"""
import os
import numpy as np
import ml_dtypes

import concourse.bass as bass
import concourse.bacc as bacc
import concourse.tile as tile
import concourse.mybir as mybir
from concourse.bass_utils import run_bass_kernel_spmd

BF16 = mybir.dt.bfloat16
F32 = mybir.dt.float32
BFNP = ml_dtypes.bfloat16

B = 2

_CACHED_NC = [None]


def _mmgroup(nc, acc, pairs, tile_position=None):
    """Issue an accumulation group of matmuls: pairs = [(lhsT, rhs), ...]."""
    n = len(pairs)
    for i, (lhsT, rhs) in enumerate(pairs):
        nc.tensor.matmul(acc, lhsT, rhs, start=(i == 0), stop=(i == n - 1),
                         tile_position=tile_position)


def _build_nc():
    if _CACHED_NC[0] is not None:
        return _CACHED_NC[0]
    nc = bacc.Bacc("TRN2", target_bir_lowering=False, debug=False, num_devices=8)
    AFT = mybir.ActivationFunctionType
    ALU = mybir.AluOpType

    dp = nc.declare_dram_parameter
    # stage A inputs
    Rd = dp("R", [100, 28, 32, 32], BF16, isOutput=False)
    m1d = dp("m1", [32, 12, 32, 32], BF16, isOutput=False)
    W1d = dp("W1T", [100, 5, 32], BF16, isOutput=False)
    # combined weights: [:, 0:25] (or 0:50 for paired) = packed-K part on 128
    # partitions; trailing 25 cols = dz=4 leftover on the first Cin partitions.
    G1Ad = dp("G1A", [128, 5, 160], BF16, isOutput=False)
    G1Bd = dp("G1B", [32, 5, 160], BF16, isOutput=False)
    W2d = dp("W2", [128, 50, 64], BF16, isOutput=False)
    G2PAd = dp("G2PA", [128, 2, 5, 128], BF16, isOutput=False)
    G2PBd = dp("G2PB", [128, 2, 5, 128], BF16, isOutput=False)
    G2E4d = dp("G2E4", [64, 5, 256], BF16, isOutput=False)
    G2Xd = dp("G2X", [128, 2, 5, 64], BF16, isOutput=False)
    G2X4d = dp("G2X4", [64, 5, 64], BF16, isOutput=False)
    W3d = dp("W3", [128, 75, 96], BF16, isOutput=False)
    G3d = dp("G3", [96, 125, 96], BF16, isOutput=False)
    W4d = dp("W4", [96, 125, 128], BF16, isOutput=False)
    # biases / betas (beta already divided by s^2)
    b1d = dp("b1", [128, 1], F32, isOutput=False)
    b2d = dp("b2", [128, 1], F32, isOutput=False)
    b3d = dp("b3", [96, 1], F32, isOutput=False)
    b4d = dp("b4", [128, 1], F32, isOutput=False)
    t1d = dp("bt1", [128, 1], F32, isOutput=False)
    t2d = dp("bt2", [128, 1], F32, isOutput=False)
    t3d = dp("bt3", [96, 1], F32, isOutput=False)
    # masks
    m2d = dp("m2", [64, 16 * 16 * 16], BF16, isOutput=False)
    m3ad = dp("m3a", [96, 512], BF16, isOutput=False)
    m3bd = dp("m3b", [128, 512], BF16, isOutput=False)
    zd = dp("zeros", [128, 12960], BF16, isOutput=False)

    outd = dp("out", [128, 512], F32, isOutput=True)

    P1 = 1296  # 36*36 plane
    P2 = 400   # 20*20 plane
    P3 = 144   # 12*12 plane
    cc1_ins = [nc.dram_tensor(f"cc1_in{i}", [32, 2 * P1], BF16) for i in range(4)]
    cc1_outs = [nc.dram_tensor(f"cc1_out{i}", [4, 32, 2 * P1], BF16) for i in range(4)]

    with tile.TileContext(nc) as tc:
        with (
            tc.tile_pool(name="persist", bufs=1) as pp,
            tc.tile_pool(name="stream", bufs=2) as st,
            tc.tile_pool(name="win", bufs=2) as wpool,
            tc.tile_pool(name="ps", bufs=8, space="PSUM") as ps,
        ):
            def load(pool, dram, shape, dtype=BF16, tag=None, eng=None):
                t = pool.tile(shape, dtype, name=dram.name + "_t", tag=tag or "")
                (eng or nc.sync).dma_start(t[:], dram[:])
                return t

            W1t = load(pp, W1d, [100, 5, 32])
            G1At = load(pp, G1Ad, [128, 5, 160])
            G1Bt = load(pp, G1Bd, [32, 5, 160])
            # R first chunks next (feeds the first matmuls)
            R0t = pp.tile([100, 14, 32, 32], BF16, name="R0t", tag="rA")
            for i in range(4):
                a, b = (0, 4, 8, 11)[i], (4, 8, 11, 14)[i]
                nc.sync.dma_start(R0t[:, a:b], Rd[:, a:b])
            R1t = pp.tile([100, 14, 32, 32], BF16, name="R1t", tag="rB")
            for i in range(4):
                a, b = (0, 4, 8, 11)[i], (4, 8, 11, 14)[i]
                nc.sync.dma_start(R1t[:, a:b], Rd[:, 14 + a : 14 + b])
            b1t = load(pp, b1d, [128, 1], F32)
            b2t = load(pp, b2d, [128, 1], F32)
            b3t = load(pp, b3d, [96, 1], F32)
            b4t = load(pp, b4d, [128, 1], F32)
            t1t = load(pp, t1d, [128, 1], F32)
            t2t = load(pp, t2d, [128, 1], F32)
            t3t = load(pp, t3d, [96, 1], F32)
            m3at = load(pp, m3ad, [96, 512])
            m3bt = load(pp, m3bd, [128, 512])

            def Rplane(zi):  # [100, 32, 32] view of input plane zi
                return (R0t if zi < 14 else R1t)[:, zi % 14, :, :]

            # XM1 packed: partition group g=zi//3 holds planes 3g..3g+3
            XM1p = pp.tile([128, 3 * P1], BF16, name="XM1p")
            nc.sync.dma_start(XM1p[:], zd[:, 0 : 3 * P1])

            def xm1_plane(zi):  # [32, 36, 36] view of slab plane zi in [0,12)
                g, r = divmod(zi, 3)
                v = XM1p[32 * g : 32 * g + 32, r * P1 : (r + 1) * P1]
                return v.rearrange("p (y x) -> p y x", y=36)

            for i in range(4):
                nc.sync.dma_start(cc1_ins[i][:], zd[0:32, 0 : 2 * P1])

            SQ1p = pp.tile([128, 3 * P1], BF16, name="SQ1p")
            def sq1_plane(zi):
                g, r = divmod(zi, 3)
                return SQ1p[32 * g : 32 * g + 32, r * P1 : (r + 1) * P1]
            REP1 = pp.tile([128, 12 * P1], BF16, name="REP1", tag="rep")
            # REP1[32g+c, t] = sq[t+g-2] = SQ1p plane u=t+g; runs keyed by the
            # last source plane they need, so each fires as soon as possible
            REP1_RUNS = {}
            for g in range(4):
                u = g
                while u < 12:
                    run = min(3 - u % 3, 12 - u)
                    REP1_RUNS.setdefault(u + run - 1, []).append((g, u, run))
                    u += run

            # ---- L1: conv1 s2, out slab zeta in [0,12) -> XM1 interior ----
            for zi in range(12):
                pb = 32 * (zi // 3)
                for h in range(2):
                    acc = ps.tile([128, 512], F32, name="acc", tag="acc")
                    _mmgroup(nc, acc[pb : pb + 32, :], [
                        (W1t[:, dz, :], Rplane(2 * zi + dz)[:, 16 * h : 16 * h + 16, :])
                        for dz in range(5)
                    ], tile_position=(0, pb))
                    m1c = st.tile([128, 512], BF16, name="m1c", tag="m1c")
                    nc.sync.dma_start(m1c[pb : pb + 32, :], m1d[:, zi, 16 * h : 16 * h + 16, :])
                    av = acc[pb : pb + 32, :].rearrange("p (y x) -> p y x", y=16)
                    nc.vector.scalar_tensor_tensor(
                        xm1_plane(zi)[:, 2 + 16 * h : 18 + 16 * h, 2:34],
                        av, b1t[pb : pb + 32, :],
                        m1c[pb : pb + 32, :].rearrange("p (y x) -> p y x", y=16),
                        ALU.add, ALU.mult,
                    )
                if True:
                    nc.vector.tensor_mul(
                        sq1_plane(zi),
                        xm1_plane(zi).rearrange("p y x -> p (y x)"),
                        xm1_plane(zi).rearrange("p y x -> p (y x)"))
                    for (g, u0, run) in REP1_RUNS.get(zi, ()):
                        nc.sync.dma_start(
                            REP1[32 * g : 32 * g + 32, (u0 - g) * P1 : (u0 - g + run) * P1],
                            SQ1p[32 * (u0 // 3) : 32 * (u0 // 3) + 32,
                                 (u0 % 3) * P1 : (u0 % 3 + run) * P1],
                        )

            # (SQ muls and REP1 runs are interleaved into the L1 loop above)
            R1view = REP1[:].rearrange("p (z y x) -> p z y x", z=12, y=36)

            W2t = load(pp, W2d, [128, 50, 64], eng=nc.gpsimd)
            G2PAt = load(pp, G2PAd, [128, 2, 5, 128], eng=nc.gpsimd)
            G2PBt = load(pp, G2PBd, [128, 2, 5, 128], eng=nc.gpsimd)
            G2E4t = load(pp, G2E4d, [64, 5, 256], eng=nc.gpsimd)
            G2Xt = load(pp, G2Xd, [128, 2, 5, 64], eng=nc.gpsimd)
            G2X4t = load(pp, G2X4d, [64, 5, 64], eng=nc.gpsimd)
            W3t = load(pp, W3d, [128, 75, 96], eng=nc.gpsimd)
            G3t = load(pp, G3d, [96, 125, 96], tag="rA", eng=nc.gpsimd)
            W4t = load(pp, W4d, [96, 125, 128], tag="rB", eng=nc.gpsimd)

            # ---- GDN1: den conv (M-packed over dx) + y1 -> cc1_in ----
            R1flat = REP1[:].rearrange("p (z c) -> p z c", z=12)
            for zi in range(8):
                pb = 32 * ((zi + 2) // 3)
                for (y0, cy) in ((0, 14), (14, 14), (28, 4)):
                    NN = cy * 36
                    pmm = ps.tile([128, 504], F32, name="pmm", tag="acc")
                    pv = pmm[:, 0:NN]
                    # matmuls sorted by (K, M) geometry: mixing weight
                    # geometries within a stream costs ~35% PE throughput
                    mms = []
                    NE = NN - 4
                    for dy in range(5):
                        row = (y0 + dy) * 36
                        mms.append((pv, G1At[:, dy, 0:128], R1flat[:, zi, row : row + NN]))
                    for dy in range(5):
                        row = (y0 + dy) * 36
                        mms.append((pv, G1Bt[:, dy, 0:128], R1flat[0:32, zi + 4, row : row + NN]))
                    for dy in range(5):
                        row = (y0 + dy) * 36
                        # edge (dx=4): contiguous flat read shifted +4; trailing
                        # cols of each psum row take garbage the combine skips
                        mms.append((pmm[0:32, 0:NE], G1At[:, dy, 128:160],
                                    R1flat[:, zi, row + 4 : row + 4 + NE]))
                    for dy in range(5):
                        row = (y0 + dy) * 36
                        mms.append((pmm[0:32, 0:NE], G1Bt[:, dy, 128:160],
                                    R1flat[0:32, zi + 4, row + 4 : row + 4 + NE]))
                    for i, (out_ap, lhsT, rhs) in enumerate(mms):
                        nc.tensor.matmul(out_ap, lhsT, rhs,
                                         start=(i == 0), stop=(i == len(mms) - 1),
                                         skip_group_check=True)
                    pvv = pv.rearrange("p (y x) -> p y x", y=cy)
                    comb = st.tile([128, 896], F32, name="comb", tag="comb")
                    s1 = comb[pb : pb + 32, 0 : cy * 32]
                    s1v = s1.rearrange("p (y x) -> p y x", y=cy)
                    nc.scalar.activation(s1v, pvv[0:32, :, 0:32], AFT.Copy)
                    nc.vector.scalar_tensor_tensor(
                        s1v, pvv[32:64, :, 1:33], 0.0, s1v, ALU.add, ALU.add)
                    nc.vector.scalar_tensor_tensor(
                        s1v, pvv[64:96, :, 2:34], 0.0, s1v, ALU.add, ALU.add)
                    nc.vector.scalar_tensor_tensor(
                        s1v, pvv[96:128, :, 3:35], t1t[pb : pb + 32, :], s1v,
                        ALU.add, ALU.add)
                    tt = st.tile([128, 512], F32, name="tt", tag="tden")
                    nc.scalar.activation(tt[pb : pb + 32, 0 : cy * 32], s1, AFT.Sqrt)
                    rr = st.tile([128, 512], F32, name="rr", tag="rden")
                    nc.vector.reciprocal(rr[pb : pb + 32, 0 : cy * 32],
                                         tt[pb : pb + 32, 0 : cy * 32])
                    y1c = st.tile([128, 512], BF16, name="y1c", tag="y1c")
                    nc.vector.tensor_mul(
                        y1c[pb : pb + 32, 0 : cy * 32].rearrange("p (y x) -> p y x", y=cy),
                        rr[pb : pb + 32, 0 : cy * 32].rearrange("p (y x) -> p y x", y=cy),
                        xm1_plane(zi + 2)[:, 2 + y0 : 2 + y0 + cy, 2:34],
                    )
                    cct = cc1_ins[zi // 2]
                    cv = cct[:].rearrange("p (z y x) -> p z y x", z=2, y=36)
                    nc.sync.dma_start(
                        cv[:, zi % 2, 2 + y0 : 2 + y0 + cy, 2:34],
                        y1c[pb : pb + 32, 0 : cy * 32].rearrange("p (y x) -> p y x", y=cy),
                    )


            # ---- AG1 (split in two so the first half overlaps GDN1) ----
            for i in range(4):
                nc.gpsimd.collective_compute(
                    "AllGather", mybir.AluOpType.bypass,
                    replica_groups=[[0, 1, 2, 3], [4, 5, 6, 7]],
                    ins=[cc1_ins[i][:]], outs=[cc1_outs[i][:]],
                )

            # ---- Stage B (full batch per core) ----
            # XM2/Y2/SQ2 packed: partition group g=zp//10 holds planes 10g..10g+10
            XM2p = pp.tile([128, 10 * P2], BF16, name="XM2p")
            nc.sync.dma_start(XM2p[:], zd[:, 0 : 10 * P2])
            Y2p = pp.tile([128, 10 * P2], BF16, name="Y2p")
            nc.sync.dma_start(Y2p[:], zd[:, 0 : 10 * P2])

            def pk2(tilep, zp, n):  # [64, n, 20, 20] view, planes zp..zp+n (same decade)
                g, r = divmod(zp, 10)
                assert r + n <= 10
                v = tilep[64 * g : 64 * g + 64, r * P2 : (r + n) * P2]
                return v.rearrange("p (z y x) -> p z y x", z=n, y=20)

            # L2: conv2 s2 from gathered y1 (windowed z planes)
            # wt[g][col d] = y1pad[2*(z2+d)+g]; wt4[0:32][col d] = y1pad[2*(z2+d)+4]
            def fill_plane(dst, pcol, zp):
                """DMA y1pad plane zp (global padded z) into dst plane column."""
                if zp < 2 or zp >= 34:
                    nc.sync.dma_start(dst[:, pcol * P1 : (pcol + 1) * P1], zd[0:32, 0:P1])
                else:
                    rank, zz = (zp - 2) // 8, (zp - 2) % 8
                    cct = cc1_outs[zz // 2]
                    nc.sync.dma_start(
                        dst[:, pcol * P1 : (pcol + 1) * P1],
                        cct[rank, :, (zz % 2) * P1 : (zz % 2 + 1) * P1],
                    )

            for z2 in range(0, 16, 2):
                wt = wpool.tile([128, 2 * P1], BF16, name="wt", tag="l2win")
                wt4 = wpool.tile([32, 2 * P1], BF16, name="wt4", tag="l2win4")
                for g in range(4):
                    for d in range(2):
                        fill_plane(wt[32 * g : 32 * g + 32], d, 2 * (z2 + d) + g)
                for d in range(2):
                    fill_plane(wt4[0:32], d, 2 * (z2 + d) + 4)
                wv = wt[:].rearrange("p (z y x) -> p z y x", z=2, y=36)
                w4v = wt4[:].rearrange("p (z y x) -> p z y x", z=2, y=36)
                pb = 64 * ((2 + z2) // 10)
                acc = ps.tile([128, 512], F32, name="acc", tag="acc")
                mms = []
                for dy in range(5):
                    for dx in range(5):
                        mms.append((
                            W2t[:, dy * 5 + dx, :],
                            wv[:, :, dy : dy + 31 : 2, dx : dx + 31 : 2],
                        ))
                for dy in range(5):
                    for dx in range(5):
                        mms.append((
                            W2t[0:32, 25 + dy * 5 + dx, :],
                            w4v[:, :, dy : dy + 31 : 2, dx : dx + 31 : 2],
                        ))
                _mmgroup(nc, acc[pb : pb + 64, :], mms, tile_position=(0, pb))
                m2c = st.tile([128, 512], BF16, name="m2c", tag="m2c")
                nc.sync.dma_start(m2c[pb : pb + 64, :], m2d[:, z2 * 256 : (z2 + 2) * 256])
                for d in range(2):
                    av = acc[pb : pb + 64, 256 * d : 256 * d + 256].rearrange(
                        "p (y x) -> p y x", y=16)
                    nc.vector.scalar_tensor_tensor(
                        pk2(XM2p, 2 + z2 + d, 1)[:, 0, 2:18, 2:18],
                        av, b2t[pb : pb + 64, :],
                        m2c[pb : pb + 64, 256 * d : 256 * d + 256].rearrange(
                            "p (y x) -> p y x", y=16),
                        ALU.add, ALU.mult,
                    )

            # SQ2 packed + replicas (pz=2)
            SQ2p = pp.tile([128, 10 * P2], BF16, name="SQ2p")
            for zp in range(20):
                g, r = divmod(zp, 10)
                nc.vector.tensor_mul(
                    SQ2p[64 * g : 64 * g + 64, r * P2 : (r + 1) * P2],
                    XM2p[64 * g : 64 * g + 64, r * P2 : (r + 1) * P2],
                    XM2p[64 * g : 64 * g + 64, r * P2 : (r + 1) * P2])

            def build_rep2(dst, srcp):
                # dst[0:64][tz] = src[tz]; dst[64:128][tz] = src[tz+1]
                for a, b in ((0, 5), (5, 10)):
                    nc.sync.dma_start(dst[0:64, a * P2 : b * P2],
                                      srcp[0:64, a * P2 : b * P2])
                    nc.sync.dma_start(dst[0:64, (10 + a) * P2 : (10 + b) * P2],
                                      srcp[64:128, a * P2 : b * P2])
                for a, b in ((0, 5), (5, 9)):
                    nc.sync.dma_start(dst[64:128, a * P2 : b * P2],
                                      srcp[0:64, (a + 1) * P2 : (b + 1) * P2])
                nc.sync.dma_start(dst[64:128, 9 * P2 : 10 * P2], srcp[64:128, 0:P2])
                for a, b in ((0, 5), (5, 9)):
                    nc.sync.dma_start(dst[64:128, (10 + a) * P2 : (10 + b) * P2],
                                      srcp[64:128, (a + 1) * P2 : (b + 1) * P2])

            REP2 = pp.tile([128, 20 * P2], BF16, name="REP2", tag="rep")
            build_rep2(REP2, SQ2p)
            R2v = REP2[:].rearrange("p (z y x) -> p z y x", z=20, y=20)

            # GDN2 (M-packed over dx pairs, full-row psum, contiguous rhs)
            R2flat = REP2[:].rearrange("p (z c) -> p z c", z=20)
            for z2 in range(16):
                pb = 64 * ((2 + z2) // 10)
                psA = ps.tile([128, 320], F32, name="psA", tag="acc")
                psB = ps.tile([128, 320], F32, name="psB", tag="acc")
                mmsA, mmsB = [], []
                for Kp in range(2):
                    for dy in range(5):
                        rhs = R2flat[:, z2 + 2 * Kp, dy * 20 : dy * 20 + 320]
                        mmsA.append((psA[:], G2PAt[:, Kp, dy, :], rhs))
                        mmsB.append((psB[:], G2PBt[:, Kp, dy, :], rhs))
                for dy in range(5):
                    rhs = R2flat[0:64, z2 + 4, dy * 20 : dy * 20 + 320]
                    mmsA.append((psA[:], G2E4t[:, dy, 0:128], rhs))
                    mmsB.append((psB[:], G2E4t[:, dy, 128:256], rhs))
                for Kp in range(2):
                    for dy in range(5):
                        rhs_e = R2flat[:, z2 + 2 * Kp, dy * 20 + 4 : dy * 20 + 320]
                        mmsA.append((psA[0:64, 0:316], G2Xt[:, Kp, dy, :], rhs_e))
                for dy in range(5):
                    rhs_e = R2flat[0:64, z2 + 4, dy * 20 + 4 : dy * 20 + 320]
                    mmsA.append((psA[0:64, 0:316], G2X4t[:, dy, :], rhs_e))
                for grp in (mmsA, mmsB):
                    for i, (out_ap, lhsT, rhs) in enumerate(grp):
                        nc.tensor.matmul(out_ap, lhsT, rhs,
                                         start=(i == 0), stop=(i == len(grp) - 1),
                                         skip_group_check=True)
                pAv = psA[:].rearrange("p (y x) -> p y x", y=16)
                pBv = psB[:].rearrange("p (y x) -> p y x", y=16)
                comb = st.tile([128, 896], F32, name="comb", tag="comb")
                s1 = comb[pb : pb + 64, 0:256]
                s1v = s1.rearrange("p (y x) -> p y x", y=16)
                nc.scalar.activation(s1v, pAv[0:64, :, 0:16], AFT.Copy)
                nc.vector.scalar_tensor_tensor(
                    s1v, pAv[64:128, :, 1:17], 0.0, s1v, ALU.add, ALU.add)
                nc.vector.scalar_tensor_tensor(
                    s1v, pBv[0:64, :, 2:18], 0.0, s1v, ALU.add, ALU.add)
                nc.vector.scalar_tensor_tensor(
                    s1v, pBv[64:128, :, 3:19], t2t[pb : pb + 64, :], s1v,
                    ALU.add, ALU.add)
                tt = st.tile([128, 512], F32, name="tt", tag="tden")
                nc.scalar.activation(tt[pb : pb + 64, 0:256], s1, AFT.Sqrt)
                rr = st.tile([128, 512], F32, name="rr", tag="rden")
                nc.vector.reciprocal(rr[pb : pb + 64, 0:256], tt[pb : pb + 64, 0:256])
                rv = rr[pb : pb + 64, 0:256].rearrange("p (y x) -> p y x", y=16)
                nc.vector.tensor_mul(
                    pk2(Y2p, 2 + z2, 1)[:, 0, 2:18, 2:18],
                    rv, pk2(XM2p, 2 + z2, 1)[:, 0, 2:18, 2:18],
                )

            # L3 replicas from Y2 (pz=2)
            REP3 = pp.tile([128, 20 * P2], BF16, name="REP3", tag="rep")
            build_rep2(REP3, Y2p)
            R3v = REP3[:].rearrange("p (z y x) -> p z y x", z=20, y=20)

            # L3: conv3 s2, full 8^3 volume in one psum tile
            acc3 = ps.tile([96, 512], F32, name="acc3", tag="acc")
            mms = []
            for p in range(2):
                for dy in range(5):
                    for dx in range(5):
                        mms.append((
                            W3t[:, p * 25 + dy * 5 + dx, :],
                            R3v[:, 2 * p : 2 * p + 15 : 2, dy : dy + 15 : 2, dx : dx + 15 : 2],
                        ))
            for dy in range(5):
                for dx in range(5):
                    mms.append((
                        W3t[0:64, 50 + dy * 5 + dx, :],
                        R3v[0:64, 4 : 4 + 15 : 2, dy : dy + 15 : 2, dx : dx + 15 : 2],
                    ))
            _mmgroup(nc, acc3[:], mms)
            XM3 = pp.tile([96, 12 * P3], BF16, name="XM3", tag="XM2p")
            nc.sync.dma_start(XM3[:], zd[0:96, 0 : 12 * P3])
            XM3v = XM3[:].rearrange("p (z y x) -> p z y x", z=12, y=12)
            for z in range(8):
                a3v = acc3[:, 64 * z : 64 * z + 64].rearrange("p (y x) -> p y x", y=8)
                m3av = m3at[:, 64 * z : 64 * z + 64].rearrange("p (y x) -> p y x", y=8)
                nc.vector.scalar_tensor_tensor(
                    XM3v[:, 2 + z, 2:10, 2:10], a3v, b3t[:], m3av, ALU.add, ALU.mult
                )

            SQ3 = pp.tile([96, 12 * P3], BF16, name="SQ3", tag="SQ2p")
            nc.vector.tensor_mul(SQ3[:], XM3[:], XM3[:])
            S3v = SQ3[:].rearrange("p (z y x) -> p z y x", z=12, y=12)

            # GDN3 (K=96, no packing)
            acc4 = ps.tile([96, 512], F32, name="acc4", tag="acc")
            mms = []
            for o in range(125):
                dz, r = divmod(o, 25)
                dy, dx = divmod(r, 5)
                mms.append((G3t[:, o, :], S3v[:, dz : dz + 8, dy : dy + 8, dx : dx + 8]))
            _mmgroup(nc, acc4[:], mms)
            tt3 = st.tile([96, 512], F32, name="tt3", tag="tden")
            nc.scalar.activation(tt3[:], acc4[:], AFT.Sqrt, bias=t3t[:])
            rr3 = st.tile([96, 512], F32, name="rr3", tag="rden")
            nc.vector.reciprocal(rr3[:], tt3[:])
            Y3 = pp.tile([96, 12 * P3], BF16, name="Y3", tag="Y2p")
            nc.sync.dma_start(Y3[:], zd[0:96, 0 : 12 * P3])
            Y3v = Y3[:].rearrange("p (z y x) -> p z y x", z=12, y=12)
            for z in range(8):
                r3v = rr3[:, 64 * z : 64 * z + 64].rearrange("p (y x) -> p y x", y=8)
                nc.vector.tensor_mul(
                    Y3v[:, 2 + z, 2:10, 2:10], r3v, XM3v[:, 2 + z, 2:10, 2:10]
                )

            # L4: conv4 s1
            acc5 = ps.tile([128, 512], F32, name="acc5", tag="acc")
            mms = []
            for o in range(125):
                dz, r = divmod(o, 25)
                dy, dx = divmod(r, 5)
                mms.append((W4t[:, o, :], Y3v[:, dz : dz + 8, dy : dy + 8, dx : dx + 8]))
            _mmgroup(nc, acc5[:], mms)
            outt = st.tile([128, 512], F32, name="outt", tag="outt", bufs=1)
            nc.vector.scalar_tensor_tensor(
                outt[:], acc5[:], b4t[:], m3bt[:], ALU.add, ALU.mult
            )
            nc.sync.dma_start(outd[:], outt[:])

    nc.compile()
    _CACHED_NC[0] = nc
    return nc


def _prep_core_inputs(c, x_feat, m1f, m2f, m3f, scale, weights):
    """Build the per-core input map. c in [0,8): batch c//4, slab c%4."""
    b, s = divmod(c, 4)
    (w1, b1, w2, b2, w3, b3, w4, b4, be1, ga1, be2, ga2, be3, ga3) = weights
    s2 = float(scale[b]) ** 2

    # R: im2col of padded input for out-z zeta in [0,12) (z1 = 8s-2+zeta)
    zp6 = np.pad(x_feat[b], ((0, 0), (6, 6), (2, 2), (2, 2)))  # [4, 76, 68, 68]
    zs = 16 * s
    sl = zp6[:, zs : zs + 28]  # [4, 28, 68, 68]
    Rarr = np.empty((4, 5, 5, 28, 32, 32), np.float32)
    for dy in range(5):
        for dx in range(5):
            Rarr[:, dy, dx] = sl[:, :, dy : dy + 63 : 2, dx : dx + 63 : 2]
    R = Rarr.reshape(100, 28, 32, 32).astype(BFNP)

    # m1 slab: z1 in [8s-2, 8s+10), replicated to 32 channels
    m1s = np.zeros((12, 32, 32), np.float32)
    lo, hi = 8 * s - 2, 8 * s + 10
    clo, chi = max(lo, 0), min(hi, 32)
    m1s[clo - lo : chi - lo] = m1f[b, clo:chi]
    m1 = np.broadcast_to(m1s, (32, 12, 32, 32)).astype(BFNP)

    def cw(w):  # [co,ci,dz,dy,dx] -> [dz, ci, dy, dx, co]
        return np.transpose(w, (2, 1, 3, 4, 0))

    W1T = np.transpose(w1, (1, 3, 4, 2, 0)).reshape(100, 5, 32).astype(BFNP)

    def packed_pairs(t, ci, co):
        # t [5, ci, 5, 5, co] -> [2*ci, 2*25, co] for dz pairs (2p, 2p+1)
        out = np.zeros((2, ci, 2, 25, co), np.float32)
        for j in range(2):
            for p in range(2):
                out[j, :, p] = t[2 * p + j].reshape(ci, 25, co)
        return out.reshape(2 * ci, 50, co)

    def combine(main, edge, ci_edge, co):
        # main [<=128, ncols, co]; edge [ci_edge, 25, co]
        ncols = main.shape[1]
        full = np.zeros((128, ncols + 25, co), np.float32)
        full[: main.shape[0], :ncols] = main
        full[:ci_edge, ncols:] = edge
        return full.astype(BFNP)

    g1 = cw(ga1 / s2)  # [5, 32, 5, 5, 32]
    G1A = np.zeros((128, 5, 160), np.float32)
    for dz in range(4):
        for dx in range(4):
            G1A[32 * dz : 32 * dz + 32, :, 32 * dx : 32 * dx + 32] = g1[dz][:, :, dx, :]
        G1A[32 * dz : 32 * dz + 32, :, 128:160] = g1[dz][:, :, 4, :]
    G1A = G1A.astype(BFNP)
    G1B = np.zeros((32, 5, 160), np.float32)
    for dx in range(4):
        G1B[:, :, 32 * dx : 32 * dx + 32] = g1[4][:, :, dx, :]
    G1B[:, :, 128:160] = g1[4][:, :, 4, :]
    G1B = G1B.astype(BFNP)

    t2 = cw(w2)
    W2main = np.concatenate([t2[dz].reshape(32, 25, 64) for dz in range(4)], axis=0)
    W2 = combine(W2main, t2[4].reshape(32, 25, 64), 32, 64)

    g2 = cw(ga2 / s2)  # [5, 64, 5, 5, 64]
    G2PA = np.zeros((128, 2, 5, 128), np.float32)
    G2PB = np.zeros((128, 2, 5, 128), np.float32)
    G2X = np.zeros((128, 2, 5, 64), np.float32)
    for jz in range(2):
        for Kp in range(2):
            dz = 2 * Kp + jz
            for jx in range(2):
                G2PA[64 * jz : 64 * jz + 64, Kp, :, 64 * jx : 64 * jx + 64] = g2[dz][:, :, jx, :]
                G2PB[64 * jz : 64 * jz + 64, Kp, :, 64 * jx : 64 * jx + 64] = g2[dz][:, :, 2 + jx, :]
            G2X[64 * jz : 64 * jz + 64, Kp, :, :] = g2[dz][:, :, 4, :]
    G2E4 = np.zeros((64, 5, 256), np.float32)
    for jx in range(2):
        G2E4[:, :, 64 * jx : 64 * jx + 64] = g2[4][:, :, jx, :]
        G2E4[:, :, 128 + 64 * jx : 192 + 64 * jx] = g2[4][:, :, 2 + jx, :]
    G2X4 = g2[4][:, :, 4, :]
    G2PA, G2PB, G2X, G2E4, G2X4 = (a.astype(BFNP) for a in (G2PA, G2PB, G2X, G2E4, G2X4))

    t3 = cw(w3)
    W3 = combine(packed_pairs(t3, 64, 96), t3[4].reshape(64, 25, 96), 64, 96)

    G3 = np.transpose(ga3 / s2, (1, 2, 3, 4, 0)).reshape(96, 125, 96).astype(BFNP)
    W4 = np.transpose(w4, (1, 2, 3, 4, 0)).reshape(96, 125, 128).astype(BFNP)

    m2 = np.broadcast_to(m2f[b].reshape(1, -1), (64, 4096)).astype(BFNP)
    m3a = np.broadcast_to(m3f[b].reshape(1, -1), (96, 512)).astype(BFNP)
    m3b = np.broadcast_to(m3f[b].reshape(1, -1), (128, 512)).astype(BFNP)

    return dict(
        R=np.ascontiguousarray(R), m1=np.ascontiguousarray(m1),
        W1T=W1T, G1A=G1A, G1B=G1B, W2=W2, W3=W3, G3=G3, W4=W4,
        G2PA=np.ascontiguousarray(G2PA), G2PB=np.ascontiguousarray(G2PB),
        G2E4=np.ascontiguousarray(G2E4), G2X=np.ascontiguousarray(G2X),
        G2X4=np.ascontiguousarray(G2X4),
        b1=np.tile(b1.reshape(32, 1), (4, 1)).astype(np.float32),
        b2=np.tile(b2.reshape(64, 1), (2, 1)).astype(np.float32),
        b3=b3.reshape(96, 1).astype(np.float32),
        b4=b4.reshape(128, 1).astype(np.float32),
        bt1=np.tile((be1 / s2).reshape(32, 1), (4, 1)).astype(np.float32),
        bt2=np.tile((be2 / s2).reshape(64, 1), (2, 1)).astype(np.float32),
        bt3=(be3 / s2).reshape(96, 1).astype(np.float32),
        m2=np.ascontiguousarray(m2), m3a=np.ascontiguousarray(m3a),
        m3b=np.ascontiguousarray(m3b),
        zeros=np.zeros((128, 12960), BFNP),
    )


def kernel(x_feat, mask, Q_F, w1, b1, w2, b2, w3, b3, w4, b4,
           beta1, gamma1, beta2, gamma2, beta3, gamma3):
    x_feat = np.asarray(x_feat, np.float32)
    maskf = np.asarray(mask)
    Q_F = np.asarray(Q_F, np.float32)
    args = [np.asarray(a, np.float32) for a in
            (w1, b1, w2, b2, w3, b3, w4, b4, beta1, gamma1, beta2, gamma2, beta3, gamma3)]
    (w1, b1, w2, b2, w3, b3, w4, b4, beta1, gamma1, beta2, gamma2, beta3, gamma3) = args

    # host-side tiny reductions (q, k) and mask pyramid
    m0 = maskf.astype(np.float32)
    q = np.mean(Q_F, axis=1)[None]          # [1, B, 2]
    scale = q[0, :, 0]                      # [B]
    c0 = m0.sum(axis=(1, 2, 3))
    m1f = m0.reshape(B, 32, 2, 32, 2, 32, 2).max(axis=(2, 4, 6))
    c1 = m1f.sum(axis=(1, 2, 3))
    m2f = m1f.reshape(B, 16, 2, 16, 2, 16, 2).max(axis=(2, 4, 6))
    c2 = m2f.sum(axis=(1, 2, 3))
    m3f = m2f.reshape(B, 8, 2, 8, 2, 8, 2).max(axis=(2, 4, 6))
    k = np.stack([c2, c1, c0]).astype(np.int32)

    weights = (w1, b1, w2, b2, w3, b3, w4, b4, beta1, gamma1, beta2, gamma2, beta3, gamma3)
    nc = _build_nc()
    in_maps = [_prep_core_inputs(c, x_feat, m1f, m2f, m3f, scale, weights)
               for c in range(8)]

    trace = os.environ.get("BASS_KERNEL_TRACE", "0") == "1"
    res = run_bass_kernel_spmd(nc, in_maps, core_ids=list(range(8)), trace=trace)
    if trace:
        kernel.last_exec_time_ns = res.exec_time_ns

    x_out = np.stack([
        res.results[0]["out"].reshape(128, 8, 8, 8),
        res.results[4]["out"].reshape(128, 8, 8, 8),
    ]).astype(np.float32)
    return x_out, q.astype(np.float32), k


kernel.last_exec_time_ns = None


# revision 22
# speedup vs baseline: 1.0526x; 1.0306x over previous
"""Trainium2 Bass kernel for nn_AnalysisTransform (3D sparse conv encoder).

Pipeline (per batch): conv5^3/s2 -> GDN -> conv/s2 -> GDN -> conv/s2 -> GDN
-> conv/s1, with voxel masks and a per-batch conditional scale.

Distribution over 8 NeuronCores: core c handles batch b=c//4, z-slab s=c%4.
Stage A (conv1 + GDN1 at 32^3) is z-sharded 4-way per batch; the GDN1 output
is AllGathered within each 4-core group; the remaining layers (16^3 and 8^3)
run replicated per group (cores 0 and 4 provide the outputs).

All matmuls run in bf16 with f32 PSUM accumulation. Convs are shift-matmuls:
contraction packs Cin x 4 z-offsets (K<=128) via z-shifted SBUF replicas;
conv1 uses host-side im2col over (dy,dx) giving K=100. The per-batch GDN
scale q[b,0] is folded into gamma/beta on the host (y = x*rsqrt((beta+den)/s^2)).

q (mean of Q_F) and k (mask voxel counts) are tiny reductions done on host.
"""
import os
import numpy as np
import ml_dtypes

import concourse.bass as bass
import concourse.bacc as bacc
import concourse.tile as tile
import concourse.mybir as mybir
from concourse.bass_utils import run_bass_kernel_spmd

BF16 = mybir.dt.bfloat16
F32 = mybir.dt.float32
BFNP = ml_dtypes.bfloat16

B = 2

_CACHED_NC = [None]


def _mmgroup(nc, acc, pairs, tile_position=None):
    """Issue an accumulation group of matmuls: pairs = [(lhsT, rhs), ...]."""
    n = len(pairs)
    for i, (lhsT, rhs) in enumerate(pairs):
        nc.tensor.matmul(acc, lhsT, rhs, start=(i == 0), stop=(i == n - 1),
                         tile_position=tile_position)


def _build_nc():
    if _CACHED_NC[0] is not None:
        return _CACHED_NC[0]
    nc = bacc.Bacc("TRN2", target_bir_lowering=False, debug=False, num_devices=8)
    AFT = mybir.ActivationFunctionType
    ALU = mybir.AluOpType

    dp = nc.declare_dram_parameter
    # stage A inputs
    Rd = dp("R", [100, 28, 32, 32], BF16, isOutput=False)
    m1d = dp("m1", [32, 12, 32, 32], BF16, isOutput=False)
    W1d = dp("W1T", [100, 5, 32], BF16, isOutput=False)
    # combined weights: [:, 0:25] (or 0:50 for paired) = packed-K part on 128
    # partitions; trailing 25 cols = dz=4 leftover on the first Cin partitions.
    G1Ad = dp("G1A", [128, 5, 160], BF16, isOutput=False)
    G1Bd = dp("G1B", [32, 5, 160], BF16, isOutput=False)
    W2d = dp("W2", [128, 50, 64], BF16, isOutput=False)
    G2PAd = dp("G2PA", [128, 2, 5, 128], BF16, isOutput=False)
    G2PBd = dp("G2PB", [128, 2, 5, 128], BF16, isOutput=False)
    G2E4d = dp("G2E4", [64, 5, 256], BF16, isOutput=False)
    G2Xd = dp("G2X", [128, 2, 5, 64], BF16, isOutput=False)
    G2X4d = dp("G2X4", [64, 5, 64], BF16, isOutput=False)
    W3d = dp("W3", [128, 75, 96], BF16, isOutput=False)
    G3d = dp("G3", [96, 125, 96], BF16, isOutput=False)
    W4d = dp("W4", [96, 125, 128], BF16, isOutput=False)
    # biases / betas (beta already divided by s^2)
    b1d = dp("b1", [128, 1], F32, isOutput=False)
    b2d = dp("b2", [128, 1], F32, isOutput=False)
    b3d = dp("b3", [96, 1], F32, isOutput=False)
    b4d = dp("b4", [128, 1], F32, isOutput=False)
    t1d = dp("bt1", [128, 1], F32, isOutput=False)
    t2d = dp("bt2", [128, 1], F32, isOutput=False)
    t3d = dp("bt3", [96, 1], F32, isOutput=False)
    # masks
    m2d = dp("m2", [64, 16 * 16 * 16], BF16, isOutput=False)
    m3ad = dp("m3a", [96, 512], BF16, isOutput=False)
    m3bd = dp("m3b", [128, 512], BF16, isOutput=False)
    zd = dp("zeros", [128, 12960], BF16, isOutput=False)

    outd = dp("out", [128, 512], F32, isOutput=True)

    P1 = 1296  # 36*36 plane
    P2 = 400   # 20*20 plane
    P3 = 144   # 12*12 plane
    cc1_ins = [nc.dram_tensor(f"cc1_in{i}", [32, 2 * P1], BF16) for i in range(4)]
    cc1_outs = [nc.dram_tensor(f"cc1_out{i}", [4, 32, 2 * P1], BF16) for i in range(4)]

    with tile.TileContext(nc) as tc:
        with (
            tc.tile_pool(name="persist", bufs=1) as pp,
            tc.tile_pool(name="stream", bufs=2) as st,
            tc.tile_pool(name="win", bufs=2) as wpool,
            tc.tile_pool(name="ps", bufs=8, space="PSUM") as ps,
        ):
            def load(pool, dram, shape, dtype=BF16, tag=None, eng=None):
                t = pool.tile(shape, dtype, name=dram.name + "_t", tag=tag or "")
                (eng or nc.sync).dma_start(t[:], dram[:])
                return t

            W1t = load(pp, W1d, [100, 5, 32])
            G1At = load(pp, G1Ad, [128, 5, 160])
            G1Bt = load(pp, G1Bd, [32, 5, 160])
            W2t = load(pp, W2d, [128, 50, 64])
            G2PAt = load(pp, G2PAd, [128, 2, 5, 128])
            G2PBt = load(pp, G2PBd, [128, 2, 5, 128])
            G2E4t = load(pp, G2E4d, [64, 5, 256])
            G2Xt = load(pp, G2Xd, [128, 2, 5, 64])
            G2X4t = load(pp, G2X4d, [64, 5, 64])
            W3t = load(pp, W3d, [128, 75, 96])
            # R first chunks next (feeds the first matmuls)
            R0t = pp.tile([100, 14, 32, 32], BF16, name="R0t", tag="rA")
            for i in range(4):
                a, b = (0, 4, 8, 11)[i], (4, 8, 11, 14)[i]
                nc.sync.dma_start(R0t[:, a:b], Rd[:, a:b])
            R1t = pp.tile([100, 14, 32, 32], BF16, name="R1t", tag="rB")
            for i in range(4):
                a, b = (0, 4, 8, 11)[i], (4, 8, 11, 14)[i]
                nc.sync.dma_start(R1t[:, a:b], Rd[:, 14 + a : 14 + b])
            b1t = load(pp, b1d, [128, 1], F32)
            b2t = load(pp, b2d, [128, 1], F32)
            b3t = load(pp, b3d, [96, 1], F32)
            b4t = load(pp, b4d, [128, 1], F32)
            t1t = load(pp, t1d, [128, 1], F32)
            t2t = load(pp, t2d, [128, 1], F32)
            t3t = load(pp, t3d, [96, 1], F32)
            m3at = load(pp, m3ad, [96, 512])
            m3bt = load(pp, m3bd, [128, 512])

            def Rplane(zi):  # [100, 32, 32] view of input plane zi
                return (R0t if zi < 14 else R1t)[:, zi % 14, :, :]

            # XM1 packed: partition group g=zi//3 holds planes 3g..3g+3
            XM1p = pp.tile([128, 3 * P1], BF16, name="XM1p")
            nc.sync.dma_start(XM1p[:], zd[:, 0 : 3 * P1])

            def xm1_plane(zi):  # [32, 36, 36] view of slab plane zi in [0,12)
                g, r = divmod(zi, 3)
                v = XM1p[32 * g : 32 * g + 32, r * P1 : (r + 1) * P1]
                return v.rearrange("p (y x) -> p y x", y=36)

            for i in range(4):
                nc.sync.dma_start(cc1_ins[i][:], zd[0:32, 0 : 2 * P1])

            SQ1p = pp.tile([128, 3 * P1], BF16, name="SQ1p")
            def sq1_plane(zi):
                g, r = divmod(zi, 3)
                return SQ1p[32 * g : 32 * g + 32, r * P1 : (r + 1) * P1]
            REP1 = pp.tile([128, 12 * P1], BF16, name="REP1", tag="rep")
            # REP1[32g+c, t] = sq[t+g-2] = SQ1p plane u=t+g; runs keyed by the
            # last source plane they need, so each fires as soon as possible
            REP1_RUNS = {}
            for g in range(4):
                u = g
                while u < 12:
                    run = min(3 - u % 3, 12 - u)
                    REP1_RUNS.setdefault(u + run - 1, []).append((g, u, run))
                    u += run

            # ---- L1: conv1 s2, out slab zeta in [0,12) -> XM1 interior ----
            for zi in range(12):
                pb = 32 * (zi // 3)
                for h in range(2):
                    acc = ps.tile([128, 512], F32, name="acc", tag="acc")
                    _mmgroup(nc, acc[pb : pb + 32, :], [
                        (W1t[:, dz, :], Rplane(2 * zi + dz)[:, 16 * h : 16 * h + 16, :])
                        for dz in range(5)
                    ], tile_position=(0, pb))
                    m1c = st.tile([128, 512], BF16, name="m1c", tag="m1c")
                    nc.sync.dma_start(m1c[pb : pb + 32, :], m1d[:, zi, 16 * h : 16 * h + 16, :])
                    av = acc[pb : pb + 32, :].rearrange("p (y x) -> p y x", y=16)
                    nc.vector.scalar_tensor_tensor(
                        xm1_plane(zi)[:, 2 + 16 * h : 18 + 16 * h, 2:34],
                        av, b1t[pb : pb + 32, :],
                        m1c[pb : pb + 32, :].rearrange("p (y x) -> p y x", y=16),
                        ALU.add, ALU.mult,
                    )
                if True:
                    nc.vector.tensor_mul(
                        sq1_plane(zi),
                        xm1_plane(zi).rearrange("p y x -> p (y x)"),
                        xm1_plane(zi).rearrange("p y x -> p (y x)"))
                    for (g, u0, run) in REP1_RUNS.get(zi, ()):
                        nc.sync.dma_start(
                            REP1[32 * g : 32 * g + 32, (u0 - g) * P1 : (u0 - g + run) * P1],
                            SQ1p[32 * (u0 // 3) : 32 * (u0 // 3) + 32,
                                 (u0 % 3) * P1 : (u0 % 3 + run) * P1],
                        )

            # (SQ muls and REP1 runs are interleaved into the L1 loop above)
            R1view = REP1[:].rearrange("p (z y x) -> p z y x", z=12, y=36)

            # ---- GDN1: den conv (M-packed over dx) + y1 -> cc1_in ----
            R1flat = REP1[:].rearrange("p (z c) -> p z c", z=12)
            for zi in range(8):
                pb = 32 * ((zi + 2) // 3)
                for (y0, cy) in ((0, 14), (14, 14), (28, 4)):
                    NN = cy * 36
                    pmm = ps.tile([128, 504], F32, name="pmm", tag="acc")
                    pv = pmm[:, 0:NN]
                    # matmuls sorted by (K, M) geometry: mixing weight
                    # geometries within a stream costs ~35% PE throughput
                    mms = []
                    NE = NN - 4
                    for dy in range(5):
                        row = (y0 + dy) * 36
                        mms.append((pv, G1At[:, dy, 0:128], R1flat[:, zi, row : row + NN]))
                    for dy in range(5):
                        row = (y0 + dy) * 36
                        mms.append((pv, G1Bt[:, dy, 0:128], R1flat[0:32, zi + 4, row : row + NN]))
                    for dy in range(5):
                        row = (y0 + dy) * 36
                        # edge (dx=4): contiguous flat read shifted +4; trailing
                        # cols of each psum row take garbage the combine skips
                        mms.append((pmm[0:32, 0:NE], G1At[:, dy, 128:160],
                                    R1flat[:, zi, row + 4 : row + 4 + NE]))
                    for dy in range(5):
                        row = (y0 + dy) * 36
                        mms.append((pmm[0:32, 0:NE], G1Bt[:, dy, 128:160],
                                    R1flat[0:32, zi + 4, row + 4 : row + 4 + NE]))
                    for i, (out_ap, lhsT, rhs) in enumerate(mms):
                        nc.tensor.matmul(out_ap, lhsT, rhs,
                                         start=(i == 0), stop=(i == len(mms) - 1),
                                         skip_group_check=True)
                    pvv = pv.rearrange("p (y x) -> p y x", y=cy)
                    comb = st.tile([128, 896], F32, name="comb", tag="comb")
                    s1 = comb[pb : pb + 32, 0 : cy * 32]
                    s1v = s1.rearrange("p (y x) -> p y x", y=cy)
                    nc.scalar.activation(s1v, pvv[0:32, :, 0:32], AFT.Copy)
                    nc.vector.scalar_tensor_tensor(
                        s1v, pvv[32:64, :, 1:33], 0.0, s1v, ALU.add, ALU.add)
                    nc.vector.scalar_tensor_tensor(
                        s1v, pvv[64:96, :, 2:34], 0.0, s1v, ALU.add, ALU.add)
                    nc.vector.scalar_tensor_tensor(
                        s1v, pvv[96:128, :, 3:35], t1t[pb : pb + 32, :], s1v,
                        ALU.add, ALU.add)
                    tt = st.tile([128, 512], F32, name="tt", tag="tden")
                    nc.scalar.activation(tt[pb : pb + 32, 0 : cy * 32], s1, AFT.Sqrt)
                    rr = st.tile([128, 512], F32, name="rr", tag="rden")
                    nc.vector.reciprocal(rr[pb : pb + 32, 0 : cy * 32],
                                         tt[pb : pb + 32, 0 : cy * 32])
                    y1c = st.tile([128, 512], BF16, name="y1c", tag="y1c")
                    nc.vector.tensor_mul(
                        y1c[pb : pb + 32, 0 : cy * 32].rearrange("p (y x) -> p y x", y=cy),
                        rr[pb : pb + 32, 0 : cy * 32].rearrange("p (y x) -> p y x", y=cy),
                        xm1_plane(zi + 2)[:, 2 + y0 : 2 + y0 + cy, 2:34],
                    )
                    cct = cc1_ins[zi // 2]
                    cv = cct[:].rearrange("p (z y x) -> p z y x", z=2, y=36)
                    nc.sync.dma_start(
                        cv[:, zi % 2, 2 + y0 : 2 + y0 + cy, 2:34],
                        y1c[pb : pb + 32, 0 : cy * 32].rearrange("p (y x) -> p y x", y=cy),
                    )


            G3t = load(pp, G3d, [96, 125, 96], tag="rA")
            W4t = load(pp, W4d, [96, 125, 128], tag="rB")

            # ---- AG1 (split in two so the first half overlaps GDN1) ----
            for i in range(4):
                nc.gpsimd.collective_compute(
                    "AllGather", mybir.AluOpType.bypass,
                    replica_groups=[[0, 1, 2, 3], [4, 5, 6, 7]],
                    ins=[cc1_ins[i][:]], outs=[cc1_outs[i][:]],
                )

            # ---- Stage B (full batch per core) ----
            # XM2/Y2/SQ2 packed: partition group g=zp//10 holds planes 10g..10g+10
            XM2p = pp.tile([128, 10 * P2], BF16, name="XM2p")
            nc.sync.dma_start(XM2p[:], zd[:, 0 : 10 * P2])
            Y2p = pp.tile([128, 10 * P2], BF16, name="Y2p")
            nc.sync.dma_start(Y2p[:], zd[:, 0 : 10 * P2])

            def pk2(tilep, zp, n):  # [64, n, 20, 20] view, planes zp..zp+n (same decade)
                g, r = divmod(zp, 10)
                assert r + n <= 10
                v = tilep[64 * g : 64 * g + 64, r * P2 : (r + n) * P2]
                return v.rearrange("p (z y x) -> p z y x", z=n, y=20)

            # L2: conv2 s2 from gathered y1 (windowed z planes)
            # wt[g][col d] = y1pad[2*(z2+d)+g]; wt4[0:32][col d] = y1pad[2*(z2+d)+4]
            def fill_plane(dst, pcol, zp):
                """DMA y1pad plane zp (global padded z) into dst plane column."""
                if zp < 2 or zp >= 34:
                    nc.sync.dma_start(dst[:, pcol * P1 : (pcol + 1) * P1], zd[0:32, 0:P1])
                else:
                    rank, zz = (zp - 2) // 8, (zp - 2) % 8
                    cct = cc1_outs[zz // 2]
                    nc.sync.dma_start(
                        dst[:, pcol * P1 : (pcol + 1) * P1],
                        cct[rank, :, (zz % 2) * P1 : (zz % 2 + 1) * P1],
                    )

            for z2 in range(0, 16, 2):
                wt = wpool.tile([128, 2 * P1], BF16, name="wt", tag="l2win")
                wt4 = wpool.tile([32, 2 * P1], BF16, name="wt4", tag="l2win4")
                for g in range(4):
                    for d in range(2):
                        fill_plane(wt[32 * g : 32 * g + 32], d, 2 * (z2 + d) + g)
                for d in range(2):
                    fill_plane(wt4[0:32], d, 2 * (z2 + d) + 4)
                wv = wt[:].rearrange("p (z y x) -> p z y x", z=2, y=36)
                w4v = wt4[:].rearrange("p (z y x) -> p z y x", z=2, y=36)
                pb = 64 * ((2 + z2) // 10)
                acc = ps.tile([128, 512], F32, name="acc", tag="acc")
                mms = []
                for dy in range(5):
                    for dx in range(5):
                        mms.append((
                            W2t[:, dy * 5 + dx, :],
                            wv[:, :, dy : dy + 31 : 2, dx : dx + 31 : 2],
                        ))
                for dy in range(5):
                    for dx in range(5):
                        mms.append((
                            W2t[0:32, 25 + dy * 5 + dx, :],
                            w4v[:, :, dy : dy + 31 : 2, dx : dx + 31 : 2],
                        ))
                _mmgroup(nc, acc[pb : pb + 64, :], mms, tile_position=(0, pb))
                m2c = st.tile([128, 512], BF16, name="m2c", tag="m2c")
                nc.sync.dma_start(m2c[pb : pb + 64, :], m2d[:, z2 * 256 : (z2 + 2) * 256])
                for d in range(2):
                    av = acc[pb : pb + 64, 256 * d : 256 * d + 256].rearrange(
                        "p (y x) -> p y x", y=16)
                    nc.vector.scalar_tensor_tensor(
                        pk2(XM2p, 2 + z2 + d, 1)[:, 0, 2:18, 2:18],
                        av, b2t[pb : pb + 64, :],
                        m2c[pb : pb + 64, 256 * d : 256 * d + 256].rearrange(
                            "p (y x) -> p y x", y=16),
                        ALU.add, ALU.mult,
                    )

            # SQ2 packed + replicas (pz=2)
            SQ2p = pp.tile([128, 10 * P2], BF16, name="SQ2p")
            for zp in range(20):
                g, r = divmod(zp, 10)
                nc.vector.tensor_mul(
                    SQ2p[64 * g : 64 * g + 64, r * P2 : (r + 1) * P2],
                    XM2p[64 * g : 64 * g + 64, r * P2 : (r + 1) * P2],
                    XM2p[64 * g : 64 * g + 64, r * P2 : (r + 1) * P2])

            def build_rep2(dst, srcp):
                # dst[0:64][tz] = src[tz]; dst[64:128][tz] = src[tz+1]
                for a, b in ((0, 5), (5, 10)):
                    nc.sync.dma_start(dst[0:64, a * P2 : b * P2],
                                      srcp[0:64, a * P2 : b * P2])
                    nc.sync.dma_start(dst[0:64, (10 + a) * P2 : (10 + b) * P2],
                                      srcp[64:128, a * P2 : b * P2])
                for a, b in ((0, 5), (5, 9)):
                    nc.sync.dma_start(dst[64:128, a * P2 : b * P2],
                                      srcp[0:64, (a + 1) * P2 : (b + 1) * P2])
                nc.sync.dma_start(dst[64:128, 9 * P2 : 10 * P2], srcp[64:128, 0:P2])
                for a, b in ((0, 5), (5, 9)):
                    nc.sync.dma_start(dst[64:128, (10 + a) * P2 : (10 + b) * P2],
                                      srcp[64:128, (a + 1) * P2 : (b + 1) * P2])

            REP2 = pp.tile([128, 20 * P2], BF16, name="REP2", tag="rep")
            build_rep2(REP2, SQ2p)
            R2v = REP2[:].rearrange("p (z y x) -> p z y x", z=20, y=20)

            # GDN2 (M-packed over dx pairs, full-row psum, contiguous rhs)
            R2flat = REP2[:].rearrange("p (z c) -> p z c", z=20)
            for z2 in range(16):
                pb = 64 * ((2 + z2) // 10)
                psA = ps.tile([128, 320], F32, name="psA", tag="acc")
                psB = ps.tile([128, 320], F32, name="psB", tag="acc")
                mmsA, mmsB = [], []
                for Kp in range(2):
                    for dy in range(5):
                        rhs = R2flat[:, z2 + 2 * Kp, dy * 20 : dy * 20 + 320]
                        mmsA.append((psA[:], G2PAt[:, Kp, dy, :], rhs))
                        mmsB.append((psB[:], G2PBt[:, Kp, dy, :], rhs))
                for dy in range(5):
                    rhs = R2flat[0:64, z2 + 4, dy * 20 : dy * 20 + 320]
                    mmsA.append((psA[:], G2E4t[:, dy, 0:128], rhs))
                    mmsB.append((psB[:], G2E4t[:, dy, 128:256], rhs))
                for Kp in range(2):
                    for dy in range(5):
                        rhs_e = R2flat[:, z2 + 2 * Kp, dy * 20 + 4 : dy * 20 + 320]
                        mmsA.append((psA[0:64, 0:316], G2Xt[:, Kp, dy, :], rhs_e))
                for dy in range(5):
                    rhs_e = R2flat[0:64, z2 + 4, dy * 20 + 4 : dy * 20 + 320]
                    mmsA.append((psA[0:64, 0:316], G2X4t[:, dy, :], rhs_e))
                for grp in (mmsA, mmsB):
                    for i, (out_ap, lhsT, rhs) in enumerate(grp):
                        nc.tensor.matmul(out_ap, lhsT, rhs,
                                         start=(i == 0), stop=(i == len(grp) - 1),
                                         skip_group_check=True)
                pAv = psA[:].rearrange("p (y x) -> p y x", y=16)
                pBv = psB[:].rearrange("p (y x) -> p y x", y=16)
                comb = st.tile([128, 896], F32, name="comb", tag="comb")
                s1 = comb[pb : pb + 64, 0:256]
                s1v = s1.rearrange("p (y x) -> p y x", y=16)
                nc.scalar.activation(s1v, pAv[0:64, :, 0:16], AFT.Copy)
                nc.vector.scalar_tensor_tensor(
                    s1v, pAv[64:128, :, 1:17], 0.0, s1v, ALU.add, ALU.add)
                nc.vector.scalar_tensor_tensor(
                    s1v, pBv[0:64, :, 2:18], 0.0, s1v, ALU.add, ALU.add)
                nc.vector.scalar_tensor_tensor(
                    s1v, pBv[64:128, :, 3:19], t2t[pb : pb + 64, :], s1v,
                    ALU.add, ALU.add)
                tt = st.tile([128, 512], F32, name="tt", tag="tden")
                nc.scalar.activation(tt[pb : pb + 64, 0:256], s1, AFT.Sqrt)
                rr = st.tile([128, 512], F32, name="rr", tag="rden")
                nc.vector.reciprocal(rr[pb : pb + 64, 0:256], tt[pb : pb + 64, 0:256])
                rv = rr[pb : pb + 64, 0:256].rearrange("p (y x) -> p y x", y=16)
                nc.vector.tensor_mul(
                    pk2(Y2p, 2 + z2, 1)[:, 0, 2:18, 2:18],
                    rv, pk2(XM2p, 2 + z2, 1)[:, 0, 2:18, 2:18],
                )

            # L3 replicas from Y2 (pz=2)
            REP3 = pp.tile([128, 20 * P2], BF16, name="REP3", tag="rep")
            build_rep2(REP3, Y2p)
            R3v = REP3[:].rearrange("p (z y x) -> p z y x", z=20, y=20)

            # L3: conv3 s2, full 8^3 volume in one psum tile
            acc3 = ps.tile([96, 512], F32, name="acc3", tag="acc")
            mms = []
            for p in range(2):
                for dy in range(5):
                    for dx in range(5):
                        mms.append((
                            W3t[:, p * 25 + dy * 5 + dx, :],
                            R3v[:, 2 * p : 2 * p + 15 : 2, dy : dy + 15 : 2, dx : dx + 15 : 2],
                        ))
            for dy in range(5):
                for dx in range(5):
                    mms.append((
                        W3t[0:64, 50 + dy * 5 + dx, :],
                        R3v[0:64, 4 : 4 + 15 : 2, dy : dy + 15 : 2, dx : dx + 15 : 2],
                    ))
            _mmgroup(nc, acc3[:], mms)
            XM3 = pp.tile([96, 12 * P3], BF16, name="XM3", tag="XM2p")
            nc.sync.dma_start(XM3[:], zd[0:96, 0 : 12 * P3])
            XM3v = XM3[:].rearrange("p (z y x) -> p z y x", z=12, y=12)
            for z in range(8):
                a3v = acc3[:, 64 * z : 64 * z + 64].rearrange("p (y x) -> p y x", y=8)
                m3av = m3at[:, 64 * z : 64 * z + 64].rearrange("p (y x) -> p y x", y=8)
                nc.vector.scalar_tensor_tensor(
                    XM3v[:, 2 + z, 2:10, 2:10], a3v, b3t[:], m3av, ALU.add, ALU.mult
                )

            SQ3 = pp.tile([96, 12 * P3], BF16, name="SQ3", tag="SQ2p")
            nc.vector.tensor_mul(SQ3[:], XM3[:], XM3[:])
            S3v = SQ3[:].rearrange("p (z y x) -> p z y x", z=12, y=12)

            # GDN3 (K=96, no packing)
            acc4 = ps.tile([96, 512], F32, name="acc4", tag="acc")
            mms = []
            for o in range(125):
                dz, r = divmod(o, 25)
                dy, dx = divmod(r, 5)
                mms.append((G3t[:, o, :], S3v[:, dz : dz + 8, dy : dy + 8, dx : dx + 8]))
            _mmgroup(nc, acc4[:], mms)
            tt3 = st.tile([96, 512], F32, name="tt3", tag="tden")
            nc.scalar.activation(tt3[:], acc4[:], AFT.Sqrt, bias=t3t[:])
            rr3 = st.tile([96, 512], F32, name="rr3", tag="rden")
            nc.vector.reciprocal(rr3[:], tt3[:])
            Y3 = pp.tile([96, 12 * P3], BF16, name="Y3", tag="Y2p")
            nc.sync.dma_start(Y3[:], zd[0:96, 0 : 12 * P3])
            Y3v = Y3[:].rearrange("p (z y x) -> p z y x", z=12, y=12)
            for z in range(8):
                r3v = rr3[:, 64 * z : 64 * z + 64].rearrange("p (y x) -> p y x", y=8)
                nc.vector.tensor_mul(
                    Y3v[:, 2 + z, 2:10, 2:10], r3v, XM3v[:, 2 + z, 2:10, 2:10]
                )

            # L4: conv4 s1
            acc5 = ps.tile([128, 512], F32, name="acc5", tag="acc")
            mms = []
            for o in range(125):
                dz, r = divmod(o, 25)
                dy, dx = divmod(r, 5)
                mms.append((W4t[:, o, :], Y3v[:, dz : dz + 8, dy : dy + 8, dx : dx + 8]))
            _mmgroup(nc, acc5[:], mms)
            outt = st.tile([128, 512], F32, name="outt", tag="outt", bufs=1)
            nc.vector.scalar_tensor_tensor(
                outt[:], acc5[:], b4t[:], m3bt[:], ALU.add, ALU.mult
            )
            nc.sync.dma_start(outd[:], outt[:])

    nc.compile()
    _CACHED_NC[0] = nc
    return nc


def _prep_core_inputs(c, x_feat, m1f, m2f, m3f, scale, weights):
    """Build the per-core input map. c in [0,8): batch c//4, slab c%4."""
    b, s = divmod(c, 4)
    (w1, b1, w2, b2, w3, b3, w4, b4, be1, ga1, be2, ga2, be3, ga3) = weights
    s2 = float(scale[b]) ** 2

    # R: im2col of padded input for out-z zeta in [0,12) (z1 = 8s-2+zeta)
    zp6 = np.pad(x_feat[b], ((0, 0), (6, 6), (2, 2), (2, 2)))  # [4, 76, 68, 68]
    zs = 16 * s
    sl = zp6[:, zs : zs + 28]  # [4, 28, 68, 68]
    Rarr = np.empty((4, 5, 5, 28, 32, 32), np.float32)
    for dy in range(5):
        for dx in range(5):
            Rarr[:, dy, dx] = sl[:, :, dy : dy + 63 : 2, dx : dx + 63 : 2]
    R = Rarr.reshape(100, 28, 32, 32).astype(BFNP)

    # m1 slab: z1 in [8s-2, 8s+10), replicated to 32 channels
    m1s = np.zeros((12, 32, 32), np.float32)
    lo, hi = 8 * s - 2, 8 * s + 10
    clo, chi = max(lo, 0), min(hi, 32)
    m1s[clo - lo : chi - lo] = m1f[b, clo:chi]
    m1 = np.broadcast_to(m1s, (32, 12, 32, 32)).astype(BFNP)

    def cw(w):  # [co,ci,dz,dy,dx] -> [dz, ci, dy, dx, co]
        return np.transpose(w, (2, 1, 3, 4, 0))

    W1T = np.transpose(w1, (1, 3, 4, 2, 0)).reshape(100, 5, 32).astype(BFNP)

    def packed_pairs(t, ci, co):
        # t [5, ci, 5, 5, co] -> [2*ci, 2*25, co] for dz pairs (2p, 2p+1)
        out = np.zeros((2, ci, 2, 25, co), np.float32)
        for j in range(2):
            for p in range(2):
                out[j, :, p] = t[2 * p + j].reshape(ci, 25, co)
        return out.reshape(2 * ci, 50, co)

    def combine(main, edge, ci_edge, co):
        # main [<=128, ncols, co]; edge [ci_edge, 25, co]
        ncols = main.shape[1]
        full = np.zeros((128, ncols + 25, co), np.float32)
        full[: main.shape[0], :ncols] = main
        full[:ci_edge, ncols:] = edge
        return full.astype(BFNP)

    g1 = cw(ga1 / s2)  # [5, 32, 5, 5, 32]
    G1A = np.zeros((128, 5, 160), np.float32)
    for dz in range(4):
        for dx in range(4):
            G1A[32 * dz : 32 * dz + 32, :, 32 * dx : 32 * dx + 32] = g1[dz][:, :, dx, :]
        G1A[32 * dz : 32 * dz + 32, :, 128:160] = g1[dz][:, :, 4, :]
    G1A = G1A.astype(BFNP)
    G1B = np.zeros((32, 5, 160), np.float32)
    for dx in range(4):
        G1B[:, :, 32 * dx : 32 * dx + 32] = g1[4][:, :, dx, :]
    G1B[:, :, 128:160] = g1[4][:, :, 4, :]
    G1B = G1B.astype(BFNP)

    t2 = cw(w2)
    W2main = np.concatenate([t2[dz].reshape(32, 25, 64) for dz in range(4)], axis=0)
    W2 = combine(W2main, t2[4].reshape(32, 25, 64), 32, 64)

    g2 = cw(ga2 / s2)  # [5, 64, 5, 5, 64]
    G2PA = np.zeros((128, 2, 5, 128), np.float32)
    G2PB = np.zeros((128, 2, 5, 128), np.float32)
    G2X = np.zeros((128, 2, 5, 64), np.float32)
    for jz in range(2):
        for Kp in range(2):
            dz = 2 * Kp + jz
            for jx in range(2):
                G2PA[64 * jz : 64 * jz + 64, Kp, :, 64 * jx : 64 * jx + 64] = g2[dz][:, :, jx, :]
                G2PB[64 * jz : 64 * jz + 64, Kp, :, 64 * jx : 64 * jx + 64] = g2[dz][:, :, 2 + jx, :]
            G2X[64 * jz : 64 * jz + 64, Kp, :, :] = g2[dz][:, :, 4, :]
    G2E4 = np.zeros((64, 5, 256), np.float32)
    for jx in range(2):
        G2E4[:, :, 64 * jx : 64 * jx + 64] = g2[4][:, :, jx, :]
        G2E4[:, :, 128 + 64 * jx : 192 + 64 * jx] = g2[4][:, :, 2 + jx, :]
    G2X4 = g2[4][:, :, 4, :]
    G2PA, G2PB, G2X, G2E4, G2X4 = (a.astype(BFNP) for a in (G2PA, G2PB, G2X, G2E4, G2X4))

    t3 = cw(w3)
    W3 = combine(packed_pairs(t3, 64, 96), t3[4].reshape(64, 25, 96), 64, 96)

    G3 = np.transpose(ga3 / s2, (1, 2, 3, 4, 0)).reshape(96, 125, 96).astype(BFNP)
    W4 = np.transpose(w4, (1, 2, 3, 4, 0)).reshape(96, 125, 128).astype(BFNP)

    m2 = np.broadcast_to(m2f[b].reshape(1, -1), (64, 4096)).astype(BFNP)
    m3a = np.broadcast_to(m3f[b].reshape(1, -1), (96, 512)).astype(BFNP)
    m3b = np.broadcast_to(m3f[b].reshape(1, -1), (128, 512)).astype(BFNP)

    return dict(
        R=np.ascontiguousarray(R), m1=np.ascontiguousarray(m1),
        W1T=W1T, G1A=G1A, G1B=G1B, W2=W2, W3=W3, G3=G3, W4=W4,
        G2PA=np.ascontiguousarray(G2PA), G2PB=np.ascontiguousarray(G2PB),
        G2E4=np.ascontiguousarray(G2E4), G2X=np.ascontiguousarray(G2X),
        G2X4=np.ascontiguousarray(G2X4),
        b1=np.tile(b1.reshape(32, 1), (4, 1)).astype(np.float32),
        b2=np.tile(b2.reshape(64, 1), (2, 1)).astype(np.float32),
        b3=b3.reshape(96, 1).astype(np.float32),
        b4=b4.reshape(128, 1).astype(np.float32),
        bt1=np.tile((be1 / s2).reshape(32, 1), (4, 1)).astype(np.float32),
        bt2=np.tile((be2 / s2).reshape(64, 1), (2, 1)).astype(np.float32),
        bt3=(be3 / s2).reshape(96, 1).astype(np.float32),
        m2=np.ascontiguousarray(m2), m3a=np.ascontiguousarray(m3a),
        m3b=np.ascontiguousarray(m3b),
        zeros=np.zeros((128, 12960), BFNP),
    )


def kernel(x_feat, mask, Q_F, w1, b1, w2, b2, w3, b3, w4, b4,
           beta1, gamma1, beta2, gamma2, beta3, gamma3):
    x_feat = np.asarray(x_feat, np.float32)
    maskf = np.asarray(mask)
    Q_F = np.asarray(Q_F, np.float32)
    args = [np.asarray(a, np.float32) for a in
            (w1, b1, w2, b2, w3, b3, w4, b4, beta1, gamma1, beta2, gamma2, beta3, gamma3)]
    (w1, b1, w2, b2, w3, b3, w4, b4, beta1, gamma1, beta2, gamma2, beta3, gamma3) = args

    # host-side tiny reductions (q, k) and mask pyramid
    m0 = maskf.astype(np.float32)
    q = np.mean(Q_F, axis=1)[None]          # [1, B, 2]
    scale = q[0, :, 0]                      # [B]
    c0 = m0.sum(axis=(1, 2, 3))
    m1f = m0.reshape(B, 32, 2, 32, 2, 32, 2).max(axis=(2, 4, 6))
    c1 = m1f.sum(axis=(1, 2, 3))
    m2f = m1f.reshape(B, 16, 2, 16, 2, 16, 2).max(axis=(2, 4, 6))
    c2 = m2f.sum(axis=(1, 2, 3))
    m3f = m2f.reshape(B, 8, 2, 8, 2, 8, 2).max(axis=(2, 4, 6))
    k = np.stack([c2, c1, c0]).astype(np.int32)

    weights = (w1, b1, w2, b2, w3, b3, w4, b4, beta1, gamma1, beta2, gamma2, beta3, gamma3)
    nc = _build_nc()
    in_maps = [_prep_core_inputs(c, x_feat, m1f, m2f, m3f, scale, weights)
               for c in range(8)]

    trace = os.environ.get("BASS_KERNEL_TRACE", "0") == "1"
    res = run_bass_kernel_spmd(nc, in_maps, core_ids=list(range(8)), trace=trace)
    if trace:
        kernel.last_exec_time_ns = res.exec_time_ns

    x_out = np.stack([
        res.results[0]["out"].reshape(128, 8, 8, 8),
        res.results[4]["out"].reshape(128, 8, 8, 8),
    ]).astype(np.float32)
    return x_out, q.astype(np.float32), k


kernel.last_exec_time_ns = None


# revision 25
# speedup vs baseline: 1.1602x; 1.1023x over previous
"""Trainium2 Bass kernel for nn_AnalysisTransform (3D sparse conv encoder).

Pipeline (per batch): conv5^3/s2 -> GDN -> conv/s2 -> GDN -> conv/s2 -> GDN
-> conv/s1, with voxel masks and a per-batch conditional scale.

Distribution over 8 NeuronCores: core c handles batch b=c//4, z-slab s=c%4.
Stage A (conv1 + GDN1 at 32^3) is z-sharded 4-way per batch; the GDN1 output
is AllGathered within each 4-core group; the remaining layers (16^3 and 8^3)
run replicated per group (cores 0 and 4 provide the outputs).

All matmuls run in bf16 with f32 PSUM accumulation. Convs are shift-matmuls:
contraction packs Cin x 4 z-offsets (K<=128) via z-shifted SBUF replicas;
conv1 uses host-side im2col over (dy,dx) giving K=100. The per-batch GDN
scale q[b,0] is folded into gamma/beta on the host (y = x*rsqrt((beta+den)/s^2)).

q (mean of Q_F) and k (mask voxel counts) are tiny reductions done on host.
"""
import os
import numpy as np
import ml_dtypes

import concourse.bass as bass
import concourse.bacc as bacc
import concourse.tile as tile
import concourse.mybir as mybir
from concourse.bass_utils import run_bass_kernel_spmd

BF16 = mybir.dt.bfloat16
F32 = mybir.dt.float32
BFNP = ml_dtypes.bfloat16

B = 2

_CACHED_NC = [None]


def _mmgroup(nc, acc, pairs, tile_position=None):
    """Issue an accumulation group of matmuls: pairs = [(lhsT, rhs), ...]."""
    n = len(pairs)
    for i, (lhsT, rhs) in enumerate(pairs):
        nc.tensor.matmul(acc, lhsT, rhs, start=(i == 0), stop=(i == n - 1),
                         tile_position=tile_position)


def _build_nc():
    if _CACHED_NC[0] is not None:
        return _CACHED_NC[0]
    nc = bacc.Bacc("TRN2", target_bir_lowering=False, debug=False, num_devices=8)
    AFT = mybir.ActivationFunctionType
    ALU = mybir.AluOpType

    dp = nc.declare_dram_parameter
    # stage A inputs
    Rd = dp("R", [100, 28, 32, 32], BF16, isOutput=False)
    m1d = dp("m1", [32, 12, 32, 32], BF16, isOutput=False)
    W1d = dp("W1T", [100, 5, 32], BF16, isOutput=False)
    # combined weights: [:, 0:25] (or 0:50 for paired) = packed-K part on 128
    # partitions; trailing 25 cols = dz=4 leftover on the first Cin partitions.
    G1Ad = dp("G1A", [128, 5, 160], BF16, isOutput=False)
    G1Bd = dp("G1B", [32, 5, 160], BF16, isOutput=False)
    W2d = dp("W2", [128, 50, 64], BF16, isOutput=False)
    G2PAd = dp("G2PA", [128, 2, 5, 128], BF16, isOutput=False)
    G2PBd = dp("G2PB", [128, 2, 5, 128], BF16, isOutput=False)
    G2E4d = dp("G2E4", [64, 5, 256], BF16, isOutput=False)
    G2Xd = dp("G2X", [128, 2, 5, 64], BF16, isOutput=False)
    G2X4d = dp("G2X4", [64, 5, 64], BF16, isOutput=False)
    W3d = dp("W3", [128, 75, 96], BF16, isOutput=False)
    G3d = dp("G3", [96, 125, 96], BF16, isOutput=False)
    W4d = dp("W4", [96, 125, 128], BF16, isOutput=False)
    # biases / betas (beta already divided by s^2)
    b1d = dp("b1", [128, 1], F32, isOutput=False)
    b2d = dp("b2", [128, 1], F32, isOutput=False)
    b3d = dp("b3", [96, 1], F32, isOutput=False)
    b4d = dp("b4", [128, 1], F32, isOutput=False)
    t1d = dp("bt1", [128, 1], F32, isOutput=False)
    t2d = dp("bt2", [128, 1], F32, isOutput=False)
    t3d = dp("bt3", [96, 1], F32, isOutput=False)
    # masks
    m2d = dp("m2", [64, 16 * 16 * 16], BF16, isOutput=False)
    m3ad = dp("m3a", [96, 512], BF16, isOutput=False)
    m3bd = dp("m3b", [128, 512], BF16, isOutput=False)
    zd = dp("zeros", [128, 12960], BF16, isOutput=False)

    outd = dp("out", [128, 512], F32, isOutput=True)

    P1 = 1296  # 36*36 plane
    P2 = 400   # 20*20 plane
    P3 = 144   # 12*12 plane
    cc1_ins = [nc.dram_tensor(f"cc1_in{i}", [32, 2 * P1], BF16) for i in range(4)]
    cc1_outs = [nc.dram_tensor(f"cc1_out{i}", [4, 32, 2 * P1], BF16) for i in range(4)]

    with tile.TileContext(nc) as tc:
        with (
            tc.tile_pool(name="persist", bufs=1) as pp,
            tc.tile_pool(name="stream", bufs=2) as st,
            tc.tile_pool(name="win", bufs=2) as wpool,
            tc.tile_pool(name="ps", bufs=8, space="PSUM") as ps,
        ):
            def load(pool, dram, shape, dtype=BF16, tag=None, eng=None):
                t = pool.tile(shape, dtype, name=dram.name + "_t", tag=tag or "")
                (eng or nc.sync).dma_start(t[:], dram[:])
                return t

            W1t = load(pp, W1d, [100, 5, 32])
            G1At = load(pp, G1Ad, [128, 5, 160])
            G1Bt = load(pp, G1Bd, [32, 5, 160])
            W2t = load(pp, W2d, [128, 50, 64], eng=nc.scalar)
            G2PAt = load(pp, G2PAd, [128, 2, 5, 128], eng=nc.scalar)
            G2PBt = load(pp, G2PBd, [128, 2, 5, 128], eng=nc.scalar)
            G2E4t = load(pp, G2E4d, [64, 5, 256], eng=nc.scalar)
            G2Xt = load(pp, G2Xd, [128, 2, 5, 64], eng=nc.scalar)
            G2X4t = load(pp, G2X4d, [64, 5, 64], eng=nc.scalar)
            W3t = load(pp, W3d, [128, 75, 96], eng=nc.scalar)
            # R first chunks next (feeds the first matmuls)
            R0t = pp.tile([100, 14, 32, 32], BF16, name="R0t", tag="rA")
            for i in range(4):
                a, b = (0, 4, 8, 11)[i], (4, 8, 11, 14)[i]
                nc.sync.dma_start(R0t[:, a:b], Rd[:, a:b])
            R1t = pp.tile([100, 14, 32, 32], BF16, name="R1t", tag="rB")
            for i in range(4):
                a, b = (0, 4, 8, 11)[i], (4, 8, 11, 14)[i]
                nc.sync.dma_start(R1t[:, a:b], Rd[:, 14 + a : 14 + b])
            b1t = load(pp, b1d, [128, 1], F32)
            b2t = load(pp, b2d, [128, 1], F32)
            b3t = load(pp, b3d, [96, 1], F32)
            b4t = load(pp, b4d, [128, 1], F32)
            t1t = load(pp, t1d, [128, 1], F32)
            t2t = load(pp, t2d, [128, 1], F32)
            t3t = load(pp, t3d, [96, 1], F32)
            m3at = load(pp, m3ad, [96, 512])
            m3bt = load(pp, m3bd, [128, 512])

            def Rplane(zi):  # [100, 32, 32] view of input plane zi
                return (R0t if zi < 14 else R1t)[:, zi % 14, :, :]

            # XM1 packed: partition group g=zi//3 holds planes 3g..3g+3
            XM1p = pp.tile([128, 3 * P1], BF16, name="XM1p")
            nc.sync.dma_start(XM1p[:], zd[:, 0 : 3 * P1])

            def xm1_plane(zi):  # [32, 36, 36] view of slab plane zi in [0,12)
                g, r = divmod(zi, 3)
                v = XM1p[32 * g : 32 * g + 32, r * P1 : (r + 1) * P1]
                return v.rearrange("p (y x) -> p y x", y=36)

            for i in range(4):
                nc.sync.dma_start(cc1_ins[i][:], zd[0:32, 0 : 2 * P1])

            SQ1p = pp.tile([128, 3 * P1], BF16, name="SQ1p")
            def sq1_plane(zi):
                g, r = divmod(zi, 3)
                return SQ1p[32 * g : 32 * g + 32, r * P1 : (r + 1) * P1]
            REP1 = pp.tile([128, 12 * P1], BF16, name="REP1", tag="rep")
            # REP1[32g+c, t] = sq[t+g-2] = SQ1p plane u=t+g; runs keyed by the
            # last source plane they need, so each fires as soon as possible
            REP1_RUNS = {}
            for g in range(4):
                u = g
                while u < 12:
                    run = min(3 - u % 3, 12 - u)
                    REP1_RUNS.setdefault(u + run - 1, []).append((g, u, run))
                    u += run

            # ---- L1: conv1 s2, out slab zeta in [0,12) -> XM1 interior ----
            for zi in range(12):
                pb = 32 * (zi // 3)
                for h in range(2):
                    acc = ps.tile([128, 512], F32, name="acc", tag="acc")
                    _mmgroup(nc, acc[pb : pb + 32, :], [
                        (W1t[:, dz, :], Rplane(2 * zi + dz)[:, 16 * h : 16 * h + 16, :])
                        for dz in range(5)
                    ], tile_position=(0, pb))
                    m1c = st.tile([128, 512], BF16, name="m1c", tag="m1c")
                    nc.sync.dma_start(m1c[pb : pb + 32, :], m1d[:, zi, 16 * h : 16 * h + 16, :])
                    av = acc[pb : pb + 32, :].rearrange("p (y x) -> p y x", y=16)
                    nc.vector.scalar_tensor_tensor(
                        xm1_plane(zi)[:, 2 + 16 * h : 18 + 16 * h, 2:34],
                        av, b1t[pb : pb + 32, :],
                        m1c[pb : pb + 32, :].rearrange("p (y x) -> p y x", y=16),
                        ALU.add, ALU.mult,
                    )
                if True:
                    nc.vector.tensor_mul(
                        sq1_plane(zi),
                        xm1_plane(zi).rearrange("p y x -> p (y x)"),
                        xm1_plane(zi).rearrange("p y x -> p (y x)"))
                    for (g, u0, run) in REP1_RUNS.get(zi, ()):
                        nc.sync.dma_start(
                            REP1[32 * g : 32 * g + 32, (u0 - g) * P1 : (u0 - g + run) * P1],
                            SQ1p[32 * (u0 // 3) : 32 * (u0 // 3) + 32,
                                 (u0 % 3) * P1 : (u0 % 3 + run) * P1],
                        )

            # (SQ muls and REP1 runs are interleaved into the L1 loop above)
            R1view = REP1[:].rearrange("p (z y x) -> p z y x", z=12, y=36)

            # ---- GDN1: den conv (M-packed over dx) + y1 -> cc1_in ----
            R1flat = REP1[:].rearrange("p (z c) -> p z c", z=12)
            for zi in range(8):
                pb = 32 * ((zi + 2) // 3)
                for (y0, cy) in ((0, 14), (14, 14), (28, 4)):
                    NN = cy * 36
                    pmm = ps.tile([128, 504], F32, name="pmm", tag="acc")
                    pv = pmm[:, 0:NN]
                    # matmuls sorted by (K, M) geometry: mixing weight
                    # geometries within a stream costs ~35% PE throughput
                    mms = []
                    NE = NN - 4
                    for dy in range(5):
                        row = (y0 + dy) * 36
                        mms.append((pv, G1At[:, dy, 0:128], R1flat[:, zi, row : row + NN]))
                    for dy in range(5):
                        row = (y0 + dy) * 36
                        mms.append((pv, G1Bt[:, dy, 0:128], R1flat[0:32, zi + 4, row : row + NN]))
                    for dy in range(5):
                        row = (y0 + dy) * 36
                        # edge (dx=4): contiguous flat read shifted +4; trailing
                        # cols of each psum row take garbage the combine skips
                        mms.append((pmm[0:32, 0:NE], G1At[:, dy, 128:160],
                                    R1flat[:, zi, row + 4 : row + 4 + NE]))
                    for dy in range(5):
                        row = (y0 + dy) * 36
                        mms.append((pmm[0:32, 0:NE], G1Bt[:, dy, 128:160],
                                    R1flat[0:32, zi + 4, row + 4 : row + 4 + NE]))
                    for i, (out_ap, lhsT, rhs) in enumerate(mms):
                        nc.tensor.matmul(out_ap, lhsT, rhs,
                                         start=(i == 0), stop=(i == len(mms) - 1),
                                         skip_group_check=True)
                    pvv = pv.rearrange("p (y x) -> p y x", y=cy)
                    comb = st.tile([128, 896], F32, name="comb", tag="comb")
                    s1 = comb[pb : pb + 32, 0 : cy * 32]
                    s1v = s1.rearrange("p (y x) -> p y x", y=cy)
                    nc.scalar.activation(s1v, pvv[0:32, :, 0:32], AFT.Copy)
                    nc.vector.scalar_tensor_tensor(
                        s1v, pvv[32:64, :, 1:33], 0.0, s1v, ALU.add, ALU.add)
                    nc.vector.scalar_tensor_tensor(
                        s1v, pvv[64:96, :, 2:34], 0.0, s1v, ALU.add, ALU.add)
                    nc.vector.scalar_tensor_tensor(
                        s1v, pvv[96:128, :, 3:35], t1t[pb : pb + 32, :], s1v,
                        ALU.add, ALU.add)
                    tt = st.tile([128, 512], F32, name="tt", tag="tden")
                    nc.scalar.activation(tt[pb : pb + 32, 0 : cy * 32], s1, AFT.Sqrt)
                    rr = st.tile([128, 512], F32, name="rr", tag="rden")
                    nc.vector.reciprocal(rr[pb : pb + 32, 0 : cy * 32],
                                         tt[pb : pb + 32, 0 : cy * 32])
                    y1c = st.tile([128, 512], BF16, name="y1c", tag="y1c", bufs=3)
                    nc.vector.tensor_mul(
                        y1c[pb : pb + 32, 0 : cy * 32].rearrange("p (y x) -> p y x", y=cy),
                        rr[pb : pb + 32, 0 : cy * 32].rearrange("p (y x) -> p y x", y=cy),
                        xm1_plane(zi + 2)[:, 2 + y0 : 2 + y0 + cy, 2:34],
                    )
                    cct = cc1_ins[zi // 2]
                    cv = cct[:].rearrange("p (z y x) -> p z y x", z=2, y=36)
                    nc.sync.dma_start(
                        cv[:, zi % 2, 2 + y0 : 2 + y0 + cy, 2:34],
                        y1c[pb : pb + 32, 0 : cy * 32].rearrange("p (y x) -> p y x", y=cy),
                    )


            G3t = load(pp, G3d, [96, 125, 96], tag="rA", eng=nc.scalar)
            W4t = load(pp, W4d, [96, 125, 128], tag="rB", eng=nc.scalar)

            # ---- AG1 (split in two so the first half overlaps GDN1) ----
            for i in range(4):
                nc.gpsimd.collective_compute(
                    "AllGather", mybir.AluOpType.bypass,
                    replica_groups=[[0, 1, 2, 3], [4, 5, 6, 7]],
                    ins=[cc1_ins[i][:]], outs=[cc1_outs[i][:]],
                )

            # ---- Stage B (full batch per core) ----
            # XM2/Y2/SQ2 packed: partition group g=zp//10 holds planes 10g..10g+10
            XM2p = pp.tile([128, 10 * P2], BF16, name="XM2p")
            nc.sync.dma_start(XM2p[:], zd[:, 0 : 10 * P2])
            Y2p = pp.tile([128, 10 * P2], BF16, name="Y2p")
            nc.sync.dma_start(Y2p[:], zd[:, 0 : 10 * P2])

            def pk2(tilep, zp, n):  # [64, n, 20, 20] view, planes zp..zp+n (same decade)
                g, r = divmod(zp, 10)
                assert r + n <= 10
                v = tilep[64 * g : 64 * g + 64, r * P2 : (r + n) * P2]
                return v.rearrange("p (z y x) -> p z y x", z=n, y=20)

            # L2: conv2 s2 from gathered y1 (windowed z planes)
            # wt[g][col d] = y1pad[2*(z2+d)+g]; wt4[0:32][col d] = y1pad[2*(z2+d)+4]
            def fill_plane(dst, pcol, zp):
                """DMA y1pad plane zp (global padded z) into dst plane column."""
                if zp < 2 or zp >= 34:
                    nc.sync.dma_start(dst[:, pcol * P1 : (pcol + 1) * P1], zd[0:32, 0:P1])
                else:
                    rank, zz = (zp - 2) // 8, (zp - 2) % 8
                    cct = cc1_outs[zz // 2]
                    nc.sync.dma_start(
                        dst[:, pcol * P1 : (pcol + 1) * P1],
                        cct[rank, :, (zz % 2) * P1 : (zz % 2 + 1) * P1],
                    )

            for z2 in range(0, 16, 2):
                wt = wpool.tile([128, 2 * P1], BF16, name="wt", tag="l2win")
                wt4 = wpool.tile([32, 2 * P1], BF16, name="wt4", tag="l2win4")
                for g in range(4):
                    for d in range(2):
                        fill_plane(wt[32 * g : 32 * g + 32], d, 2 * (z2 + d) + g)
                for d in range(2):
                    fill_plane(wt4[0:32], d, 2 * (z2 + d) + 4)
                wv = wt[:].rearrange("p (z y x) -> p z y x", z=2, y=36)
                w4v = wt4[:].rearrange("p (z y x) -> p z y x", z=2, y=36)
                pb = 64 * ((2 + z2) // 10)
                acc = ps.tile([128, 512], F32, name="acc", tag="acc")
                mms = []
                for dy in range(5):
                    for dx in range(5):
                        mms.append((
                            W2t[:, dy * 5 + dx, :],
                            wv[:, :, dy : dy + 31 : 2, dx : dx + 31 : 2],
                        ))
                for dy in range(5):
                    for dx in range(5):
                        mms.append((
                            W2t[0:32, 25 + dy * 5 + dx, :],
                            w4v[:, :, dy : dy + 31 : 2, dx : dx + 31 : 2],
                        ))
                _mmgroup(nc, acc[pb : pb + 64, :], mms, tile_position=(0, pb))
                m2c = st.tile([128, 512], BF16, name="m2c", tag="m2c")
                nc.sync.dma_start(m2c[pb : pb + 64, :], m2d[:, z2 * 256 : (z2 + 2) * 256])
                for d in range(2):
                    av = acc[pb : pb + 64, 256 * d : 256 * d + 256].rearrange(
                        "p (y x) -> p y x", y=16)
                    nc.vector.scalar_tensor_tensor(
                        pk2(XM2p, 2 + z2 + d, 1)[:, 0, 2:18, 2:18],
                        av, b2t[pb : pb + 64, :],
                        m2c[pb : pb + 64, 256 * d : 256 * d + 256].rearrange(
                            "p (y x) -> p y x", y=16),
                        ALU.add, ALU.mult,
                    )

            # SQ2 packed + replicas (pz=2)
            SQ2p = pp.tile([128, 10 * P2], BF16, name="SQ2p")
            for zp in range(20):
                g, r = divmod(zp, 10)
                nc.vector.tensor_mul(
                    SQ2p[64 * g : 64 * g + 64, r * P2 : (r + 1) * P2],
                    XM2p[64 * g : 64 * g + 64, r * P2 : (r + 1) * P2],
                    XM2p[64 * g : 64 * g + 64, r * P2 : (r + 1) * P2])

            def build_rep2(dst, srcp):
                # dst[0:64][tz] = src[tz]; dst[64:128][tz] = src[tz+1]
                for a, b in ((0, 5), (5, 10)):
                    nc.sync.dma_start(dst[0:64, a * P2 : b * P2],
                                      srcp[0:64, a * P2 : b * P2])
                    nc.sync.dma_start(dst[0:64, (10 + a) * P2 : (10 + b) * P2],
                                      srcp[64:128, a * P2 : b * P2])
                for a, b in ((0, 5), (5, 9)):
                    nc.sync.dma_start(dst[64:128, a * P2 : b * P2],
                                      srcp[0:64, (a + 1) * P2 : (b + 1) * P2])
                nc.sync.dma_start(dst[64:128, 9 * P2 : 10 * P2], srcp[64:128, 0:P2])
                for a, b in ((0, 5), (5, 9)):
                    nc.sync.dma_start(dst[64:128, (10 + a) * P2 : (10 + b) * P2],
                                      srcp[64:128, (a + 1) * P2 : (b + 1) * P2])

            REP2 = pp.tile([128, 20 * P2], BF16, name="REP2", tag="rep")
            build_rep2(REP2, SQ2p)
            R2v = REP2[:].rearrange("p (z y x) -> p z y x", z=20, y=20)

            # GDN2 (M-packed over dx pairs, full-row psum, contiguous rhs)
            R2flat = REP2[:].rearrange("p (z c) -> p z c", z=20)
            for z2 in range(16):
                pb = 64 * ((2 + z2) // 10)
                psA = ps.tile([128, 320], F32, name="psA", tag="acc")
                psB = ps.tile([128, 320], F32, name="psB", tag="acc")
                mmsA, mmsB = [], []
                for Kp in range(2):
                    for dy in range(5):
                        rhs = R2flat[:, z2 + 2 * Kp, dy * 20 : dy * 20 + 320]
                        mmsA.append((psA[:], G2PAt[:, Kp, dy, :], rhs))
                        mmsB.append((psB[:], G2PBt[:, Kp, dy, :], rhs))
                for dy in range(5):
                    rhs = R2flat[0:64, z2 + 4, dy * 20 : dy * 20 + 320]
                    mmsA.append((psA[:], G2E4t[:, dy, 0:128], rhs))
                    mmsB.append((psB[:], G2E4t[:, dy, 128:256], rhs))
                for Kp in range(2):
                    for dy in range(5):
                        rhs_e = R2flat[:, z2 + 2 * Kp, dy * 20 + 4 : dy * 20 + 320]
                        mmsA.append((psA[0:64, 0:316], G2Xt[:, Kp, dy, :], rhs_e))
                for dy in range(5):
                    rhs_e = R2flat[0:64, z2 + 4, dy * 20 + 4 : dy * 20 + 320]
                    mmsA.append((psA[0:64, 0:316], G2X4t[:, dy, :], rhs_e))
                for grp in (mmsA, mmsB):
                    for i, (out_ap, lhsT, rhs) in enumerate(grp):
                        nc.tensor.matmul(out_ap, lhsT, rhs,
                                         start=(i == 0), stop=(i == len(grp) - 1),
                                         skip_group_check=True)
                pAv = psA[:].rearrange("p (y x) -> p y x", y=16)
                pBv = psB[:].rearrange("p (y x) -> p y x", y=16)
                comb = st.tile([128, 896], F32, name="comb", tag="comb")
                s1 = comb[pb : pb + 64, 0:256]
                s1v = s1.rearrange("p (y x) -> p y x", y=16)
                nc.scalar.activation(s1v, pAv[0:64, :, 0:16], AFT.Copy)
                nc.vector.scalar_tensor_tensor(
                    s1v, pAv[64:128, :, 1:17], 0.0, s1v, ALU.add, ALU.add)
                nc.vector.scalar_tensor_tensor(
                    s1v, pBv[0:64, :, 2:18], 0.0, s1v, ALU.add, ALU.add)
                nc.vector.scalar_tensor_tensor(
                    s1v, pBv[64:128, :, 3:19], t2t[pb : pb + 64, :], s1v,
                    ALU.add, ALU.add)
                tt = st.tile([128, 512], F32, name="tt", tag="tden")
                nc.scalar.activation(tt[pb : pb + 64, 0:256], s1, AFT.Sqrt)
                rr = st.tile([128, 512], F32, name="rr", tag="rden")
                nc.vector.reciprocal(rr[pb : pb + 64, 0:256], tt[pb : pb + 64, 0:256])
                rv = rr[pb : pb + 64, 0:256].rearrange("p (y x) -> p y x", y=16)
                nc.vector.tensor_mul(
                    pk2(Y2p, 2 + z2, 1)[:, 0, 2:18, 2:18],
                    rv, pk2(XM2p, 2 + z2, 1)[:, 0, 2:18, 2:18],
                )

            # L3 replicas from Y2 (pz=2)
            REP3 = pp.tile([128, 20 * P2], BF16, name="REP3", tag="rep")
            build_rep2(REP3, Y2p)
            R3v = REP3[:].rearrange("p (z y x) -> p z y x", z=20, y=20)

            # L3: conv3 s2, full 8^3 volume in one psum tile
            acc3 = ps.tile([96, 512], F32, name="acc3", tag="acc")
            mms = []
            for p in range(2):
                for dy in range(5):
                    for dx in range(5):
                        mms.append((
                            W3t[:, p * 25 + dy * 5 + dx, :],
                            R3v[:, 2 * p : 2 * p + 15 : 2, dy : dy + 15 : 2, dx : dx + 15 : 2],
                        ))
            for dy in range(5):
                for dx in range(5):
                    mms.append((
                        W3t[0:64, 50 + dy * 5 + dx, :],
                        R3v[0:64, 4 : 4 + 15 : 2, dy : dy + 15 : 2, dx : dx + 15 : 2],
                    ))
            _mmgroup(nc, acc3[:], mms)
            XM3 = pp.tile([96, 12 * P3], BF16, name="XM3", tag="XM2p")
            nc.sync.dma_start(XM3[:], zd[0:96, 0 : 12 * P3])
            XM3v = XM3[:].rearrange("p (z y x) -> p z y x", z=12, y=12)
            for z in range(8):
                a3v = acc3[:, 64 * z : 64 * z + 64].rearrange("p (y x) -> p y x", y=8)
                m3av = m3at[:, 64 * z : 64 * z + 64].rearrange("p (y x) -> p y x", y=8)
                nc.vector.scalar_tensor_tensor(
                    XM3v[:, 2 + z, 2:10, 2:10], a3v, b3t[:], m3av, ALU.add, ALU.mult
                )

            SQ3 = pp.tile([96, 12 * P3], BF16, name="SQ3", tag="SQ2p")
            nc.vector.tensor_mul(SQ3[:], XM3[:], XM3[:])
            S3v = SQ3[:].rearrange("p (z y x) -> p z y x", z=12, y=12)

            # GDN3 (K=96, no packing)
            acc4 = ps.tile([96, 512], F32, name="acc4", tag="acc")
            mms = []
            for o in range(125):
                dz, r = divmod(o, 25)
                dy, dx = divmod(r, 5)
                mms.append((G3t[:, o, :], S3v[:, dz : dz + 8, dy : dy + 8, dx : dx + 8]))
            _mmgroup(nc, acc4[:], mms)
            tt3 = st.tile([96, 512], F32, name="tt3", tag="tden")
            nc.scalar.activation(tt3[:], acc4[:], AFT.Sqrt, bias=t3t[:])
            rr3 = st.tile([96, 512], F32, name="rr3", tag="rden")
            nc.vector.reciprocal(rr3[:], tt3[:])
            Y3 = pp.tile([96, 12 * P3], BF16, name="Y3", tag="Y2p")
            nc.sync.dma_start(Y3[:], zd[0:96, 0 : 12 * P3])
            Y3v = Y3[:].rearrange("p (z y x) -> p z y x", z=12, y=12)
            for z in range(8):
                r3v = rr3[:, 64 * z : 64 * z + 64].rearrange("p (y x) -> p y x", y=8)
                nc.vector.tensor_mul(
                    Y3v[:, 2 + z, 2:10, 2:10], r3v, XM3v[:, 2 + z, 2:10, 2:10]
                )

            # L4: conv4 s1
            acc5 = ps.tile([128, 512], F32, name="acc5", tag="acc")
            mms = []
            for o in range(125):
                dz, r = divmod(o, 25)
                dy, dx = divmod(r, 5)
                mms.append((W4t[:, o, :], Y3v[:, dz : dz + 8, dy : dy + 8, dx : dx + 8]))
            _mmgroup(nc, acc5[:], mms)
            outt = st.tile([128, 512], F32, name="outt", tag="outt", bufs=1)
            nc.vector.scalar_tensor_tensor(
                outt[:], acc5[:], b4t[:], m3bt[:], ALU.add, ALU.mult
            )
            nc.sync.dma_start(outd[:], outt[:])

    nc.compile()
    _CACHED_NC[0] = nc
    return nc


def _prep_core_inputs(c, x_feat, m1f, m2f, m3f, scale, weights):
    """Build the per-core input map. c in [0,8): batch c//4, slab c%4."""
    b, s = divmod(c, 4)
    (w1, b1, w2, b2, w3, b3, w4, b4, be1, ga1, be2, ga2, be3, ga3) = weights
    s2 = float(scale[b]) ** 2

    # R: im2col of padded input for out-z zeta in [0,12) (z1 = 8s-2+zeta)
    zp6 = np.pad(x_feat[b], ((0, 0), (6, 6), (2, 2), (2, 2)))  # [4, 76, 68, 68]
    zs = 16 * s
    sl = zp6[:, zs : zs + 28]  # [4, 28, 68, 68]
    Rarr = np.empty((4, 5, 5, 28, 32, 32), np.float32)
    for dy in range(5):
        for dx in range(5):
            Rarr[:, dy, dx] = sl[:, :, dy : dy + 63 : 2, dx : dx + 63 : 2]
    R = Rarr.reshape(100, 28, 32, 32).astype(BFNP)

    # m1 slab: z1 in [8s-2, 8s+10), replicated to 32 channels
    m1s = np.zeros((12, 32, 32), np.float32)
    lo, hi = 8 * s - 2, 8 * s + 10
    clo, chi = max(lo, 0), min(hi, 32)
    m1s[clo - lo : chi - lo] = m1f[b, clo:chi]
    m1 = np.broadcast_to(m1s, (32, 12, 32, 32)).astype(BFNP)

    def cw(w):  # [co,ci,dz,dy,dx] -> [dz, ci, dy, dx, co]
        return np.transpose(w, (2, 1, 3, 4, 0))

    W1T = np.transpose(w1, (1, 3, 4, 2, 0)).reshape(100, 5, 32).astype(BFNP)

    def packed_pairs(t, ci, co):
        # t [5, ci, 5, 5, co] -> [2*ci, 2*25, co] for dz pairs (2p, 2p+1)
        out = np.zeros((2, ci, 2, 25, co), np.float32)
        for j in range(2):
            for p in range(2):
                out[j, :, p] = t[2 * p + j].reshape(ci, 25, co)
        return out.reshape(2 * ci, 50, co)

    def combine(main, edge, ci_edge, co):
        # main [<=128, ncols, co]; edge [ci_edge, 25, co]
        ncols = main.shape[1]
        full = np.zeros((128, ncols + 25, co), np.float32)
        full[: main.shape[0], :ncols] = main
        full[:ci_edge, ncols:] = edge
        return full.astype(BFNP)

    g1 = cw(ga1 / s2)  # [5, 32, 5, 5, 32]
    G1A = np.zeros((128, 5, 160), np.float32)
    for dz in range(4):
        for dx in range(4):
            G1A[32 * dz : 32 * dz + 32, :, 32 * dx : 32 * dx + 32] = g1[dz][:, :, dx, :]
        G1A[32 * dz : 32 * dz + 32, :, 128:160] = g1[dz][:, :, 4, :]
    G1A = G1A.astype(BFNP)
    G1B = np.zeros((32, 5, 160), np.float32)
    for dx in range(4):
        G1B[:, :, 32 * dx : 32 * dx + 32] = g1[4][:, :, dx, :]
    G1B[:, :, 128:160] = g1[4][:, :, 4, :]
    G1B = G1B.astype(BFNP)

    t2 = cw(w2)
    W2main = np.concatenate([t2[dz].reshape(32, 25, 64) for dz in range(4)], axis=0)
    W2 = combine(W2main, t2[4].reshape(32, 25, 64), 32, 64)

    g2 = cw(ga2 / s2)  # [5, 64, 5, 5, 64]
    G2PA = np.zeros((128, 2, 5, 128), np.float32)
    G2PB = np.zeros((128, 2, 5, 128), np.float32)
    G2X = np.zeros((128, 2, 5, 64), np.float32)
    for jz in range(2):
        for Kp in range(2):
            dz = 2 * Kp + jz
            for jx in range(2):
                G2PA[64 * jz : 64 * jz + 64, Kp, :, 64 * jx : 64 * jx + 64] = g2[dz][:, :, jx, :]
                G2PB[64 * jz : 64 * jz + 64, Kp, :, 64 * jx : 64 * jx + 64] = g2[dz][:, :, 2 + jx, :]
            G2X[64 * jz : 64 * jz + 64, Kp, :, :] = g2[dz][:, :, 4, :]
    G2E4 = np.zeros((64, 5, 256), np.float32)
    for jx in range(2):
        G2E4[:, :, 64 * jx : 64 * jx + 64] = g2[4][:, :, jx, :]
        G2E4[:, :, 128 + 64 * jx : 192 + 64 * jx] = g2[4][:, :, 2 + jx, :]
    G2X4 = g2[4][:, :, 4, :]
    G2PA, G2PB, G2X, G2E4, G2X4 = (a.astype(BFNP) for a in (G2PA, G2PB, G2X, G2E4, G2X4))

    t3 = cw(w3)
    W3 = combine(packed_pairs(t3, 64, 96), t3[4].reshape(64, 25, 96), 64, 96)

    G3 = np.transpose(ga3 / s2, (1, 2, 3, 4, 0)).reshape(96, 125, 96).astype(BFNP)
    W4 = np.transpose(w4, (1, 2, 3, 4, 0)).reshape(96, 125, 128).astype(BFNP)

    m2 = np.broadcast_to(m2f[b].reshape(1, -1), (64, 4096)).astype(BFNP)
    m3a = np.broadcast_to(m3f[b].reshape(1, -1), (96, 512)).astype(BFNP)
    m3b = np.broadcast_to(m3f[b].reshape(1, -1), (128, 512)).astype(BFNP)

    return dict(
        R=np.ascontiguousarray(R), m1=np.ascontiguousarray(m1),
        W1T=W1T, G1A=G1A, G1B=G1B, W2=W2, W3=W3, G3=G3, W4=W4,
        G2PA=np.ascontiguousarray(G2PA), G2PB=np.ascontiguousarray(G2PB),
        G2E4=np.ascontiguousarray(G2E4), G2X=np.ascontiguousarray(G2X),
        G2X4=np.ascontiguousarray(G2X4),
        b1=np.tile(b1.reshape(32, 1), (4, 1)).astype(np.float32),
        b2=np.tile(b2.reshape(64, 1), (2, 1)).astype(np.float32),
        b3=b3.reshape(96, 1).astype(np.float32),
        b4=b4.reshape(128, 1).astype(np.float32),
        bt1=np.tile((be1 / s2).reshape(32, 1), (4, 1)).astype(np.float32),
        bt2=np.tile((be2 / s2).reshape(64, 1), (2, 1)).astype(np.float32),
        bt3=(be3 / s2).reshape(96, 1).astype(np.float32),
        m2=np.ascontiguousarray(m2), m3a=np.ascontiguousarray(m3a),
        m3b=np.ascontiguousarray(m3b),
        zeros=np.zeros((128, 12960), BFNP),
    )


def kernel(x_feat, mask, Q_F, w1, b1, w2, b2, w3, b3, w4, b4,
           beta1, gamma1, beta2, gamma2, beta3, gamma3):
    x_feat = np.asarray(x_feat, np.float32)
    maskf = np.asarray(mask)
    Q_F = np.asarray(Q_F, np.float32)
    args = [np.asarray(a, np.float32) for a in
            (w1, b1, w2, b2, w3, b3, w4, b4, beta1, gamma1, beta2, gamma2, beta3, gamma3)]
    (w1, b1, w2, b2, w3, b3, w4, b4, beta1, gamma1, beta2, gamma2, beta3, gamma3) = args

    # host-side tiny reductions (q, k) and mask pyramid
    m0 = maskf.astype(np.float32)
    q = np.mean(Q_F, axis=1)[None]          # [1, B, 2]
    scale = q[0, :, 0]                      # [B]
    c0 = m0.sum(axis=(1, 2, 3))
    m1f = m0.reshape(B, 32, 2, 32, 2, 32, 2).max(axis=(2, 4, 6))
    c1 = m1f.sum(axis=(1, 2, 3))
    m2f = m1f.reshape(B, 16, 2, 16, 2, 16, 2).max(axis=(2, 4, 6))
    c2 = m2f.sum(axis=(1, 2, 3))
    m3f = m2f.reshape(B, 8, 2, 8, 2, 8, 2).max(axis=(2, 4, 6))
    k = np.stack([c2, c1, c0]).astype(np.int32)

    weights = (w1, b1, w2, b2, w3, b3, w4, b4, beta1, gamma1, beta2, gamma2, beta3, gamma3)
    nc = _build_nc()
    in_maps = [_prep_core_inputs(c, x_feat, m1f, m2f, m3f, scale, weights)
               for c in range(8)]

    trace = os.environ.get("BASS_KERNEL_TRACE", "0") == "1"
    res = run_bass_kernel_spmd(nc, in_maps, core_ids=list(range(8)), trace=trace)
    if trace:
        kernel.last_exec_time_ns = res.exec_time_ns

    x_out = np.stack([
        res.results[0]["out"].reshape(128, 8, 8, 8),
        res.results[4]["out"].reshape(128, 8, 8, 8),
    ]).astype(np.float32)
    return x_out, q.astype(np.float32), k


kernel.last_exec_time_ns = None
